# revision 1
# baseline (speedup 1.0000x reference)
"""DeepseekV2 MLA decode attention on 8 Trainium2 NeuronCores.

Strategy (single SPMD launch, identical program on all cores; all per-core
variation comes from in_maps contents and collective semantics):

  - Attention is batch-sharded: core k owns sequences 4k..4k+4, whose KV
    cache slices are fed to it via in_maps in TWO host-prepared layouts:
    natural [s, c] (context matmul, contracts s) and transposed [c, s]
    (score matmul, contracts c). The PE contracts along partitions, so the
    two matmuls need different partition assignments of the same data;
    host-side dual layout avoids all on-chip cache transposition.
  - Scores are computed transposed (PSUM [128 s, 16 h]) so the exp (ACT)
    writes e_T directly in the layout the context matmul consumes as its
    stationary operand.
  - w_qkv_a is K-sharded (hidden dim / 8); the row-major partial qkv
    activations are ReduceScattered, which both sums the partials and
    hands each core exactly its 4 sequences (rank-dependent slicing via
    collective semantics).
  - w_o is column-sharded; ctx_v rows are AllGathered and each core
    produces a 640-column slice of the output, concatenated on host.
  - q_a_norm_w is folded into w_q_b on the host (rmsnorm scale is diag).
  - The current-token cache update (rmsnorm latent / roped k_pe written
    at slot S-1) is applied on the host while building the cache layouts.
"""

import sys

sys.path.insert(0, "/opt/trn_rl_repo")

import numpy as np

import concourse.bacc as bacc
import concourse.mybir as mybir
import concourse.tile as tile
from concourse import bass_utils
from concourse.masks import make_identity

F32 = mybir.dt.float32
ADD = mybir.AluOpType.add
MULT = mybir.AluOpType.mult
BYPASS = mybir.AluOpType.bypass
EXP = mybir.ActivationFunctionType.Exp
SQRT = mybir.ActivationFunctionType.Sqrt
AXIS_X = mybir.AxisListType.X

B, HID, H = 32, 5120, 16
DN, DR, DV = 128, 64, 128
QL, KL = 1536, 512
BASE = 10000.0
EPS = 1e-6
SCALE = float((DN + DR) ** -0.5)

N_CORES = 8
BP = B // N_CORES      # sequences per core
NKT = QL // 128        # 12
TP = True              # collective-based weight sharding

_CACHE = {}


# ----------------------------- host math ---------------------------------


def _rmsnorm_np(x, w):
    ms = np.mean(x * x, axis=-1, keepdims=True, dtype=np.float32)
    return (x * (1.0 / np.sqrt(ms + EPS)) * w).astype(np.float32)


def _rope_np(x, pos):
    d = x.shape[-1]
    inv = (1.0 / (BASE ** (np.arange(0, d, 2, dtype=np.float32) / d))).astype(
        np.float32
    )
    fr = pos.astype(np.float32)[:, None] * inv
    cos, sin = np.cos(fr).astype(np.float32), np.sin(fr).astype(np.float32)
    out = np.empty_like(x)
    out[..., 0::2] = x[..., 0::2] * cos - x[..., 1::2] * sin
    out[..., 1::2] = x[..., 1::2] * cos + x[..., 0::2] * sin
    return out.astype(np.float32)


def _rope_RT(pos):
    """Per-batch transposed rotation matrices (lhsT for rope-as-matmul)."""
    inv = (1.0 / (BASE ** (np.arange(0, DR, 2, dtype=np.float32) / DR))).astype(
        np.float32
    )
    fr = pos.astype(np.float32)[:, None] * inv
    cos, sin = np.cos(fr).astype(np.float32), np.sin(fr).astype(np.float32)
    R = np.zeros((B, DR, DR), np.float32)
    j = np.arange(DR // 2)
    bi = np.arange(B)[:, None]
    R[bi, 2 * j, 2 * j] = cos
    R[bi, 2 * j, 2 * j + 1] = -sin
    R[bi, 2 * j + 1, 2 * j] = sin
    R[bi, 2 * j + 1, 2 * j + 1] = cos
    return np.ascontiguousarray(R.transpose(0, 2, 1))


# ----------------------------- device program ----------------------------


def _build(S, n_cores, tp, fake_coll=False, TRF=2):
    nc = bacc.Bacc("TRN2", target_bir_lowering=False, debug=False,
                   enable_asserts=False, num_devices=n_cores)
    ST = S // 512
    rg = [list(range(n_cores))]
    NB = B if tp else BP            # batch width of the qkv_a projection
    KTH = (HID // n_cores if tp else HID) // 128   # hidden k-tiles (5 / 40)
    HO = HID // n_cores if tp else HID             # output columns per core

    hT = nc.dram_tensor("hT", [128 * KTH, NB], F32, kind="ExternalInput")
    w_qa = nc.dram_tensor("w_qa", [128 * KTH, QL], F32, kind="ExternalInput")
    w_qb = nc.dram_tensor("w_qb", [QL, H * (DN + DR)], F32,
                          kind="ExternalInput")
    w_kc = nc.dram_tensor("w_kc", [H, DN, KL], F32, kind="ExternalInput")
    w_vc = nc.dram_tensor("w_vc", [H, KL, DV], F32, kind="ExternalInput")
    w_o = nc.dram_tensor("w_o", [H * DV, HO], F32, kind="ExternalInput")
    cache_nat = nc.dram_tensor("cache_nat", [BP, S, KL], F32,
                               kind="ExternalInput")
    cacheT_l = nc.dram_tensor("cacheT_l", [BP, KL, S], F32,
                              kind="ExternalInput")
    cacheT_r = nc.dram_tensor("cacheT_r", [BP, DR, S], F32,
                              kind="ExternalInput")
    ropeRT = nc.dram_tensor("ropeRT", [BP, DR, DR], F32, kind="ExternalInput")
    out = nc.dram_tensor("out", [NB if tp else BP, HO], F32,
                         kind="ExternalOutput")

    with tile.TileContext(nc) as tc:
        with (
            tc.tile_pool(name="const", bufs=1) as cp,
            tc.tile_pool(name="qsb", bufs=1) as qsb,
            tc.tile_pool(name="dram", bufs=1, space="DRAM") as dramp,
            tc.tile_pool(name="wstream", bufs=2) as wsp,
            tc.tile_pool(name="wo", bufs=1) as wop,
            tc.tile_pool(name="ctl", bufs=3) as ctlp,
            tc.tile_pool(name="ctr", bufs=1) as ctrp,
            tc.tile_pool(name="nat", bufs=4) as natp,
            tc.tile_pool(name="et", bufs=4) as etp,
            tc.tile_pool(name="small", bufs=1) as smp,
            tc.tile_pool(name="small2", bufs=2) as smp2,
        ):
            ones_col = cp.tile([128, 1], F32)
            nc.any.memset(ones_col, 1.0)
            eps_t = cp.tile([128, 1], F32)
            nc.any.memset(eps_t, EPS)
            ident = cp.tile([128, 128], F32)
            make_identity(nc, ident[:, :])
            rt_sb = cp.tile([DR, BP, DR], F32)
            nc.sync.dma_start(rt_sb[:, :, :],
                              ropeRT[:, :, :].rearrange("b k m -> k b m"))
            hT_sb = cp.tile([128, KTH, NB], F32)
            nc.sync.dma_start(hT_sb[:, :, :],
                              hT[:, :].rearrange("(t p) b -> p t b", p=128))

            # ================= q path =================
            with tc.tile_pool(name="psq", bufs=6, space="PSUM") as psq:

                def qps(name):
                    return psq.tile([128, 512], F32, tag="q", name=name)

                # ---- qkv_a projection: q_a rows [NB, 1536] ----
                qkv_rows = qsb.tile([NB, QL], F32)
                pss = [qps(f"qkv{j}") for j in range(3)]
                for kt in range(KTH):
                    wt = wsp.tile([128, 1536], F32, tag="wqa")
                    nc.sync.dma_start(wt[:, :],
                                      w_qa[kt * 128:(kt + 1) * 128, :])
                    for j in range(3):
                        nc.tensor.matmul(
                            pss[j][:NB, :], hT_sb[:, kt, :],
                            wt[:, j * 512:(j + 1) * 512],
                            start=(kt == 0), stop=(kt == KTH - 1))
                for j in range(3):
                    nc.any.tensor_copy(
                        qkv_rows[:, j * 512:(j + 1) * 512], pss[j][:NB, :])

                # ---- ReduceScatter partials -> my 4 sequences' q_a ----
                if tp:
                    rs_in = dramp.tile([B, QL], F32)
                    rs_out = dramp.tile([BP, QL], F32)
                    nc.sync.dma_start(rs_in[:, :], qkv_rows[:, :])
                    if fake_coll:
                        nc.sync.dma_start(rs_out[:, :], rs_in[0:BP, :])
                    else:
                        nc.gpsimd.collective_compute(
                            "ReduceScatter", ADD, replica_groups=rg,
                            ins=[rs_in.opt()], outs=[rs_out.opt()])
                    qa_mine = qsb.tile([BP, QL], F32)
                    nc.sync.dma_start(qa_mine[:, :], rs_out[:, :])
                else:
                    qa_mine = qkv_rows

                # ---- rmsnorm (rows) + transpose to [128, 12, 4] ----
                sq = smp.tile([BP, QL], F32, tag="sq")
                nc.vector.tensor_tensor(sq[:, :], qa_mine[:, :],
                                        qa_mine[:, :], MULT)
                ssum = smp.tile([BP, 1], F32, tag="ssum")
                nc.vector.reduce_sum(ssum[:, :], sq[:, :], AXIS_X)
                rms = smp.tile([BP, 1], F32, tag="rms")
                nc.scalar.activation(rms[:, :], ssum[:, :], SQRT,
                                     bias=eps_t[:BP, :1], scale=1.0 / QL)
                rinv = smp.tile([BP, 1], F32, tag="rinv")
                nc.vector.reciprocal(rinv[:, :], rms[:, :])
                qan = smp.tile([BP, QL], F32, tag="qan")
                nc.vector.tensor_scalar_mul(qan[:, :], qa_mine[:, :],
                                            rinv[:, :1])

                ps_t = qps("qanT")
                for t in range(NKT):
                    nc.tensor.transpose(ps_t[:, t * BP:(t + 1) * BP],
                                        qan[:BP, t * 128:(t + 1) * 128],
                                        ident[:BP, :BP])
                qanT = qsb.tile([128, NKT, BP], F32)
                nc.any.tensor_copy(qanT[:, :, :], ps_t[:, :NKT * BP])

                # ---- q_b (norm weight folded in) per head -> nope/pe ----
                ps_n = qps("qbn")
                ps_p = qps("qbp")
                for h in range(H):
                    wt = wsp.tile([128, NKT, DN + DR], F32, tag="wqb")
                    nc.sync.dma_start(
                        wt[:, :, :],
                        w_qb[:, h * (DN + DR):(h + 1) * (DN + DR)]
                        .rearrange("(t p) m -> p t m", p=128))
                    for t in range(NKT):
                        nc.tensor.matmul(ps_n[:, h * BP:(h + 1) * BP],
                                         wt[:, t, :DN], qanT[:, t, :],
                                         start=(t == 0), stop=(t == NKT - 1))
                    for t in range(NKT):
                        nc.tensor.matmul(ps_p[:64, h * BP:(h + 1) * BP],
                                         wt[:, t, DN:], qanT[:, t, :],
                                         start=(t == 0), stop=(t == NKT - 1))
                qnopeT = qsb.tile([128, H, BP], F32)
                nc.any.tensor_copy(qnopeT[:, :, :],
                                   ps_n[:, :H * BP]
                                   .rearrange("p (h b) -> p h b", h=H))
                qpe_raw = qsb.tile([64, H, BP], F32)
                nc.any.tensor_copy(qpe_raw[:, :, :],
                                   ps_p[:64, :H * BP]
                                   .rearrange("p (h b) -> p h b", h=H))

                # ---- rope(q_pe) as matmul with per-batch rotation ----
                ps_r = qps("rope")
                for h in range(H):
                    for b in range(BP):
                        nc.tensor.matmul(
                            ps_r[:64, h * BP + b:h * BP + b + 1],
                            rt_sb[:, b, :], qpe_raw[:, h, b:b + 1],
                            start=True, stop=True)
                qpeT = qsb.tile([64, H, BP], F32)
                nc.any.tensor_copy(qpeT[:, :, :],
                                   ps_r[:64, :H * BP]
                                   .rearrange("p (h b) -> p h b", h=H))

                # ---- absorb q_nope through w_kc: qabsT [128, 4, H, BP] ----
                ps_a = [qps(f"abs{c}") for c in range(4)]
                for h in range(H):
                    kt_ = wsp.tile([128, KL], F32, tag="wkc")
                    nc.sync.dma_start(kt_[:, :], w_kc[h, :, :])
                    for c in range(4):
                        nc.tensor.matmul(ps_a[c][:, h * BP:(h + 1) * BP],
                                         kt_[:, c * 128:(c + 1) * 128],
                                         qnopeT[:, h, :],
                                         start=True, stop=True)
                qabsT = qsb.tile([128, 4, H, BP], F32)
                for c in range(4):
                    nc.any.tensor_copy(qabsT[:, c, :, :],
                                       ps_a[c][:, :H * BP]
                                       .rearrange("p (h b) -> p h b", h=H))

            # ================= attention =================
            wvc_res = qsb.tile([128, H, 4, DV], F32)
            nc.sync.dma_start(
                wvc_res[:, :, :, :],
                w_vc[:, :, :].rearrange("h (c p) v -> p h c v", p=128))
            ctxT = qsb.tile([128, 4, H, BP], F32)
            with (
                tc.tile_pool(name="pssc", bufs=2, space="PSUM") as pssc,
                tc.tile_pool(name="psctx", bufs=2, space="PSUM") as psctx,
                tc.tile_pool(name="pssum", bufs=1, space="PSUM") as pssum,
                tc.tile_pool(name="psctt", bufs=1, space="PSUM") as psctt,
                tc.tile_pool(name="pstr", bufs=2, space="PSUM") as pstr,
            ):
                sums = pssum.tile([16, BP], F32, tag="sums")

                def attn_seq(lb, ctx_ps):
                    seq_ctr = [None]
                    for st in range(ST):
                        s0 = st * 512
                        ctl = ctlp.tile([128, 4, 512], F32, tag="ctl")
                        nc.sync.dma_start(
                            ctl[:, :, :],
                            cacheT_l[lb, :, s0:s0 + 512]
                            .rearrange("(t p) s -> p t s", p=128))
                        if st == 0:
                            ctr_seq = ctrp.tile([64, S], F32, tag="ctr")
                            nc.sync.dma_start(ctr_seq[:, :],
                                              cacheT_r[lb, :, :])
                            seq_ctr[0] = ctr_seq
                        ctr = seq_ctr[0][:, s0:s0 + 512]
                        sc = pssc.tile([128, 4 * H], F32, tag="sc")
                        for i in range(4):
                            for c in range(4):
                                nc.tensor.matmul(
                                    sc[:, i * H:(i + 1) * H],
                                    ctl[:, c, i * 128:(i + 1) * 128],
                                    qabsT[:, c, :, lb],
                                    start=(c == 0), stop=False)
                            nc.tensor.matmul(
                                sc[:, i * H:(i + 1) * H],
                                ctr[:, i * 128:(i + 1) * 128],
                                qpeT[:, :, lb], start=False, stop=True)
                        eT = etp.tile([128, 4 * H], F32, tag="eT")
                        nc.scalar.activation(eT[:, :], sc[:, :], EXP,
                                             scale=SCALE)
                        for i in range(4):
                            # natural-layout chunk: PE-transpose the resident
                            # [c, s] tile for TRF of 4 chunks, stream the
                            # rest from the host natural layout -- balances
                            # the HBM-read saving against PE transpose cost
                            natc = natp.tile([128, KL], F32, tag="nat")
                            if i < TRF:
                                ps_tr = pstr.tile([128, KL], F32, tag="tr")
                                for c in range(4):
                                    nc.tensor.transpose(
                                        ps_tr[:, c * 128:(c + 1) * 128],
                                        ctl[:, c, i * 128:(i + 1) * 128],
                                        ident[:, :])
                                nc.scalar.copy(natc[:, :], ps_tr[:, :])
                            else:
                                nc.sync.dma_start(
                                    natc[:, :],
                                    cache_nat[lb,
                                              s0 + i * 128:s0 + (i + 1) * 128,
                                              :])
                            nc.tensor.matmul(
                                ctx_ps[:16, :], eT[:, i * H:(i + 1) * H],
                                natc[:, :],
                                start=(st == 0 and i == 0),
                                stop=(st == ST - 1 and i == 3))
                            nc.tensor.matmul(
                                sums[:16, lb:lb + 1],
                                eT[:, i * H:(i + 1) * H], ones_col[:, :1],
                                start=(st == 0 and i == 0),
                                stop=(st == ST - 1 and i == 3))

                for lb in range(BP):
                    ctx_ps = psctx.tile([16, KL], F32, tag="ctx",
                                        name=f"ctx{lb}")
                    attn_seq(lb, ctx_ps)
                    rec = smp2.tile([16, 1], F32, tag="rec")
                    nc.vector.reciprocal(rec[:, :], sums[:16, lb:lb + 1])
                    ctxn = smp2.tile([16, KL], F32, tag="ctxn")
                    nc.vector.tensor_scalar_mul(ctxn[:, :], ctx_ps[:16, :],
                                                rec[:, :1])
                    ps_ct = psctt.tile([128, 4 * H], F32, tag="ctxT")
                    for c in range(4):
                        nc.tensor.transpose(ps_ct[:, c * H:(c + 1) * H],
                                            ctxn[:16, c * 128:(c + 1) * 128],
                                            ident[:16, :16])
                    nc.any.tensor_copy(
                        ctxT[:, :, :, lb],
                        ps_ct[:, :].rearrange("p (c h) -> p c h", c=4))

                # ---- un-absorb values: ovT [128 v, H, BP] ----
                ps_v = pssum.tile([128, H * BP], F32, tag="sums",
                                  name="ps_v")
                for h in range(H):
                    for c in range(4):
                        nc.tensor.matmul(ps_v[:, h * BP:(h + 1) * BP],
                                         wvc_res[:, h, c, :], ctxT[:, c, h, :],
                                         start=(c == 0), stop=(c == 3))
                ovT = qsb.tile([128, H, BP], F32)
                nc.any.tensor_copy(ovT[:, :, :],
                                   ps_v[:, :]
                                   .rearrange("p (h b) -> p h b", h=H))

            # ================= output projection =================
            with (
                tc.tile_pool(name="psor", bufs=1, space="PSUM") as psor,
                tc.tile_pool(name="psot", bufs=1, space="PSUM") as psot,
                tc.tile_pool(name="psoo", bufs=2, space="PSUM") as psoo,
            ):
                if tp:
                    # ovT -> rows [4, 2048] -> AllGather -> [32, 2048] -> T
                    ps_rows = psor.tile([BP, H * DV], F32, tag="ovr")
                    for h in range(H):
                        nc.tensor.transpose(
                            ps_rows[:BP, h * DV:(h + 1) * DV],
                            ovT[:, h, :], ident[:, :])
                    ov_rows = smp.tile([BP, H * DV], F32, tag="ovrows")
                    nc.any.tensor_copy(ov_rows[:, :], ps_rows[:BP, :])
                    agv_in = dramp.tile([BP, H * DV], F32)
                    agv_out = dramp.tile([B, H * DV], F32)
                    nc.sync.dma_start(agv_in[:, :], ov_rows[:, :])
                    if fake_coll:
                        nc.sync.dma_start(agv_out[0:BP, :], agv_in[:, :])
                    else:
                        nc.gpsimd.collective_compute(
                            "AllGather", BYPASS, replica_groups=rg,
                            ins=[agv_in.opt()], outs=[agv_out.opt()])
                    ov32 = smp.tile([B, H * DV], F32, tag="ov32")
                    nc.sync.dma_start(ov32[:, :], agv_out[:, :])
                    ps_tt = psot.tile([128, 16 * B], F32, tag="ovtt")
                    for kt in range(16):
                        nc.tensor.transpose(
                            ps_tt[:, kt * B:(kt + 1) * B],
                            ov32[:B, kt * 128:(kt + 1) * 128], ident[:B, :B])
                    ovT_f = qsb.tile([128, 16, B], F32)
                    nc.any.tensor_copy(ovT_f[:, :, :],
                                       ps_tt[:, :]
                                       .rearrange("p (k b) -> p k b", k=16))
                    lhs_o, NBO = ovT_f, B
                else:
                    lhs_o, NBO = ovT, BP

                out_sb = qsb.tile([NBO, HO], F32)
                for n0 in range(0, HO, 512):
                    nn = min(512, HO - n0)
                    wo_t = wop.tile([128, 16, 512], F32, tag="wo")
                    nc.sync.dma_start(
                        wo_t[:, :, :nn],
                        w_o[:, n0:n0 + nn]
                        .rearrange("(t p) n -> p t n", p=128))
                    ps_o = psoo.tile([NBO, 512], F32, tag="oproj")
                    for kt in range(16):
                        nc.tensor.matmul(ps_o[:, :nn], lhs_o[:, kt, :],
                                         wo_t[:, kt, :nn],
                                         start=(kt == 0), stop=(kt == 15))
                    nc.any.tensor_copy(out_sb[:, n0:n0 + nn], ps_o[:, :nn])
                nc.sync.dma_start(out[:, :], out_sb[:, :])

    nc.compile()
    return nc


# ----------------------------- host wrapper ------------------------------


def _prep_in_maps(inputs, S, n_cores, tp):
    hidden = np.asarray(inputs["hidden_states"], np.float32)
    pos = np.asarray(inputs["positions"], np.int32)
    w_qkv_a = np.asarray(inputs["w_qkv_a"], np.float32)
    q_a_norm_w = np.asarray(inputs["q_a_norm_w"], np.float32)
    w_q_b = np.asarray(inputs["w_q_b"], np.float32)
    kv_a_norm_w = np.asarray(inputs["kv_a_norm_w"], np.float32)
    w_kc = np.asarray(inputs["w_kc"], np.float32)
    w_vc = np.asarray(inputs["w_vc"], np.float32)
    w_o = np.asarray(inputs["w_o"], np.float32)
    cache_l = np.asarray(inputs["kv_cache_latent"], np.float32)
    cache_r = np.asarray(inputs["kv_cache_rope"], np.float32)

    # current-token cache update (host)
    latent = hidden @ w_qkv_a[:, QL:QL + KL]
    k_pe = hidden @ w_qkv_a[:, QL + KL:]
    latent_n = _rmsnorm_np(latent, kv_a_norm_w)
    k_pe_r = _rope_np(k_pe.astype(np.float32), pos)
    cache_l = cache_l.copy()
    cache_r = cache_r.copy()
    cache_l[:, -1, :] = latent_n
    cache_r[:, -1, :] = k_pe_r
    cacheT_l = np.ascontiguousarray(cache_l.transpose(0, 2, 1))
    cacheT_r = np.ascontiguousarray(cache_r.transpose(0, 2, 1))

    hiddenT = np.ascontiguousarray(hidden.T)
    w_qb_eff = np.ascontiguousarray(q_a_norm_w[:, None] * w_q_b)
    RT = _rope_RT(pos)
    w_qa_q = np.ascontiguousarray(w_qkv_a[:, :QL])

    in_maps = []
    for k in range(n_cores):
        b0 = k * BP
        if tp:
            k0 = k * (HID // n_cores)
            k1 = (k + 1) * (HID // n_cores)
            m = {
                "hT": np.ascontiguousarray(hiddenT[k0:k1, :]),
                "w_qa": np.ascontiguousarray(w_qa_q[k0:k1, :]),
                "w_o": np.ascontiguousarray(
                    w_o[:, k * (HID // n_cores):(k + 1) * (HID // n_cores)]),
            }
        else:
            m = {
                "hT": np.ascontiguousarray(hiddenT[:, b0:b0 + BP]),
                "w_qa": w_qa_q,
                "w_o": np.ascontiguousarray(w_o),
            }
        m.update({
            "w_qb": w_qb_eff,
            "w_kc": np.ascontiguousarray(w_kc),
            "w_vc": np.ascontiguousarray(w_vc),
            "cache_nat": np.ascontiguousarray(cache_l[b0:b0 + BP, :S, :]),
            "cacheT_l": np.ascontiguousarray(cacheT_l[b0:b0 + BP, :, :S]),
            "cacheT_r": np.ascontiguousarray(cacheT_r[b0:b0 + BP, :, :S]),
            "ropeRT": np.ascontiguousarray(RT[b0:b0 + BP]),
        })
        in_maps.append(m)
    return in_maps


def _unshard(results, tp):
    if tp:
        return np.concatenate([results[k]["out"] for k in range(N_CORES)],
                              axis=1)
    return np.concatenate([results[k]["out"] for k in range(N_CORES)], axis=0)


def run(inputs, S=4096, trace=False):
    key = (S, N_CORES, TP)
    if key not in _CACHE:
        _CACHE[key] = _build(S, N_CORES, TP)
    nc = _CACHE[key]
    in_maps = _prep_in_maps(inputs, S, N_CORES, TP)
    res = bass_utils.run_bass_kernel_spmd(
        nc, in_maps, core_ids=list(range(N_CORES)), trace=trace)
    return _unshard(res.results, TP), res


def kernel(**inputs) -> np.ndarray:
    out, _ = run(inputs)
    return out.astype(np.float32)



# revision 26
# speedup vs baseline: 2.4365x; 2.4365x over previous
"""DeepseekV2 MLA decode attention on 8 Trainium2 NeuronCores.

bf16 redesign. Strategy (single SPMD launch; per-core variation comes only
from in_maps contents and collective semantics):

  - Attention is batch-sharded: core k owns sequences 4k..4k+4. The latent KV
    cache is host-packed in ONE transposed bf16 layout [c, s]; the natural
    [s, c] tiles the context matmul needs are produced on-chip by PE
    transposes (bf16: 1 cycle/row) whose PSUM->SBUF copies are spread across
    DVE / ACT / GpSimd.  This reads the 16.8 MB/core cache exactly once.
  - Scores are computed as [s, 16h] PSUM tiles (cache tile stationary,
    absorbed-q moving, 16 columns/matmul); exp runs on ACT straight out of
    PSUM into a bf16 e^T tile; context is computed transposed ([c, h] out,
    natural tile stationary, e^T moving) so only 16 columns stream per
    matmul and the result lands directly in the layout w_vc consumes.
  - Rope is folded into the cache on the host: the rope-cache is pre-rotated
    by R(pos_b)^T per sequence, so the device does no rope at all.
  - q path: w_qkv_a's q columns are column-sharded (each core computes
    q_a[:, its 192 cols] for all 32 seqs); rmsnorm needs only an AllReduce of
    the 32 per-row sums of squares (128 B); w_kc and the q_a norm scale are
    folded into w_q_b on the host giving W2 [1536, 9216], K-sharded 192
    rows/core; the row-major partial q_abs [32, 9216] is ReduceScattered,
    which both sums partials and hands each core its 4 sequences.
  - w_o is column-sharded; ov rows are AllGathered (bf16) and each core
    produces a 640-column slice, concatenated on the host.
"""

import sys

sys.path.insert(0, "/opt/trn_rl_repo")

import ml_dtypes
import numpy as np

import concourse.bacc as bacc
import concourse.mybir as mybir
import concourse.tile as tile
from concourse import bass_utils
from concourse.masks import make_identity

F32 = mybir.dt.float32
BF16 = mybir.dt.bfloat16
ADD = mybir.AluOpType.add
MULT = mybir.AluOpType.mult
BYPASS = mybir.AluOpType.bypass
EXP = mybir.ActivationFunctionType.Exp
LN = mybir.ActivationFunctionType.Ln
SQUARE = mybir.ActivationFunctionType.Square

B, HID, H = 32, 5120, 16
DN, DR, DV = 128, 64, 128
QL, KL = 1536, 512
BASE = 10000.0
EPS = 1e-6
SCALE = float((DN + DR) ** -0.5)

N_CORES = 8
BP = B // N_CORES            # sequences per core
QLP = QL // N_CORES          # W2 contraction rows per core (192)
KTH = HID // 128             # hidden k-tiles (40)
NOPE = H * KL                # 8192 absorbed-nope columns of W2
NQ = NOPE + H * DR           # 9216 total W2 columns
HO = HID // N_CORES          # output columns per core (640)
NCH = NQ // 512              # n-chunks of the W2 row matmul (18)
TP = True                    # kept for test.py signature compat

BF = ml_dtypes.bfloat16

_CACHE = {}


# ----------------------------- host math ---------------------------------


def _rmsnorm_np(x, w):
    ms = np.mean(x * x, axis=-1, keepdims=True, dtype=np.float32)
    return (x * (1.0 / np.sqrt(ms + EPS)) * w).astype(np.float32)


def _rope_np(x, pos):
    d = x.shape[-1]
    inv = (1.0 / (BASE ** (np.arange(0, d, 2, dtype=np.float32) / d))).astype(
        np.float32
    )
    fr = pos.astype(np.float32)[:, None] * inv
    cos, sin = np.cos(fr).astype(np.float32), np.sin(fr).astype(np.float32)
    out = np.empty_like(x)
    out[..., 0::2] = x[..., 0::2] * cos - x[..., 1::2] * sin
    out[..., 1::2] = x[..., 1::2] * cos + x[..., 0::2] * sin
    return out.astype(np.float32)


# ----------------------------- device program ----------------------------


def _build(S, n_cores, tp=True, fake_coll=False, PRE=8, CPY=(3, 2),
           debug=False):
    nc = bacc.Bacc("TRN2", target_bir_lowering=False, debug=False,
                   enable_asserts=False, num_devices=n_cores)
    ST = S // 512
    rg = [list(range(n_cores))]
    ncpy = sum(CPY)

    hT = nc.dram_tensor("hT", [128, KTH, B], BF16, kind="ExternalInput")
    wqa = nc.dram_tensor("wqa", [128, KTH, QLP], BF16, kind="ExternalInput")
    w2a = nc.dram_tensor("w2a", [128, NQ], BF16, kind="ExternalInput")
    w2b = nc.dram_tensor("w2b", [QLP - 128, NQ], BF16, kind="ExternalInput")
    ctl_d = nc.dram_tensor("ctl", [BP, 128, 4, S], BF16, kind="ExternalInput")
    ctr_d = nc.dram_tensor("ctr", [BP, DR, S], BF16, kind="ExternalInput")
    wvc = nc.dram_tensor("wvc", [128, H * 4, DV], BF16, kind="ExternalInput")
    wo = nc.dram_tensor("wo", [128, H, HO], BF16, kind="ExternalInput")
    out = nc.dram_tensor("out", [HO, B], F32, kind="ExternalOutput")
    if debug:
        dbg_qr4 = nc.dram_tensor("dbg_qr4", [BP, NQ], BF16,
                                 kind="ExternalOutput")
        dbg_ctxT = nc.dram_tensor("dbg_ctxT", [128, 4, H, BP], BF16,
                                  kind="ExternalOutput")
        dbg_et = nc.dram_tensor("dbg_et", [128, S // 128, H], BF16,
                                kind="ExternalOutput")
        dbg_ovr = nc.dram_tensor("dbg_ovr", [BP, H * DV], BF16,
                                 kind="ExternalOutput")
        dbg_nat = nc.dram_tensor("dbg_nat", [8, 128, 512], BF16,
                                 kind="ExternalOutput")

    with tile.TileContext(nc) as tc:
        with (
            tc.tile_pool(name="const", bufs=1) as cp,
            tc.tile_pool(name="wq", bufs=1) as wqp,
            tc.tile_pool(name="qwork", bufs=1) as qwp,
            tc.tile_pool(name="dram", bufs=1, space="DRAM") as dramp,
            tc.tile_pool(name="ctl", bufs=PRE) as ctlp,
            tc.tile_pool(name="ctr", bufs=2) as ctrp,
            tc.tile_pool(name="natc", bufs=8) as natp,
            tc.tile_pool(name="et", bufs=2) as etp,
            tc.tile_pool(name="attn", bufs=1) as atp,
            tc.tile_pool(name="osb", bufs=1) as osb,
        ):
            # ---- constants ----
            ident = cp.tile([128, 128], BF16)
            make_identity(nc, ident[:, :])
            ones_c_bf = cp.tile([128, 1], BF16)
            nc.any.memset(ones_c_bf, 1.0)
            ones_c_f = cp.tile([128, 1], F32)
            nc.any.memset(ones_c_f, 1.0)
            ones_r_f = cp.tile([1, 128], F32)
            nc.any.memset(ones_r_f, 1.0)
            eps_t = cp.tile([1, 1], F32)
            nc.any.memset(eps_t, EPS)

            # ---- q-path weights ----
            hT_sb = wqp.tile([128, KTH, B], BF16)
            nc.sync.dma_start(hT_sb[:, :, :], hT[:, :, :])
            wqa_sb = wqp.tile([128, KTH, QLP], BF16)
            nc.sync.dma_start(wqa_sb[:, :, :], wqa[:, :, :])
            w2a_sb = wqp.tile([128, NQ], BF16)
            nc.sync.dma_start(w2a_sb[:, :], w2a[:, :])
            w2b_sb = wqp.tile([QLP - 128, NQ], BF16)
            nc.sync.dma_start(w2b_sb[:, :], w2b[:, :])

            # ---- cache prefetch (before the q-path collective DMAs so the
            # DMA engine stays busy during the collective latency) ----
            ctl_tiles = {}

            def issue_ctl(lb, st):
                t = ctlp.tile([128, 4, 512], BF16, tag="ctl")
                nc.sync.dma_start(t[:, :, :],
                                  ctl_d[lb, :, :, st * 512:(st + 1) * 512])
                ctl_tiles[(lb, st)] = t

            ctr_tiles = {}

            def issue_ctr(lb):
                t = ctrp.tile([DR, S], BF16, tag="ctr")
                nc.sync.dma_start(t[:, :], ctr_d[lb, :, :])
                ctr_tiles[lb] = t

            issue_ctr(0)
            for m in range(PRE - 2):
                issue_ctl(m // ST, m % ST)

            # ================= q path =================
            qabsT = qwp.tile([128, H * 4, BP], BF16)
            qpeT = qwp.tile([DR, H, BP], BF16)

            with tc.tile_pool(name="psqa", bufs=1, space="PSUM") as psqa:
                # q_a^T for my 192 columns, all 32 seqs
                ps_qa0 = psqa.tile([128, B], F32, tag="qa0")
                ps_qa1 = psqa.tile([QLP - 128, B], F32, tag="qa1")
                for kt in range(KTH):
                    nc.tensor.matmul(ps_qa0[:, :], wqa_sb[:, kt, 0:128],
                                     hT_sb[:, kt, :],
                                     start=(kt == 0), stop=(kt == KTH - 1))
                    nc.tensor.matmul(ps_qa1[:, :], wqa_sb[:, kt, 128:QLP],
                                     hT_sb[:, kt, :],
                                     start=(kt == 0), stop=(kt == KTH - 1))

                # local sum of squares -> AllReduce -> rinv row
                sq0 = qwp.tile([128, B], F32, tag="sq0")
                nc.scalar.activation(sq0[:, :], ps_qa0[:, :], SQUARE)
                sq1 = qwp.tile([QLP - 128, B], F32, tag="sq1")
                nc.scalar.activation(sq1[:, :], ps_qa1[:, :], SQUARE)
                ps_ss = psqa.tile([1, B], F32, tag="ss")
                nc.tensor.matmul(ps_ss[:, :], ones_c_f[:, :], sq0[:, :],
                                 start=True, stop=False)
                nc.tensor.matmul(ps_ss[:, :], ones_c_f[:QLP - 128, :],
                                 sq1[:, :], start=False, stop=True)
                ss_sb = qwp.tile([1, B], F32, tag="ss_sb")
                nc.vector.tensor_copy(ss_sb[:, :], ps_ss[:, :])
                ar_in = dramp.tile([1, B], F32)
                ar_out = dramp.tile([1, B], F32)
                nc.sync.dma_start(ar_in[:, :], ss_sb[:, :])
                if fake_coll:
                    nc.sync.dma_start(ar_out[:, :], ar_in[:, :])
                else:
                    nc.gpsimd.collective_compute(
                        "AllReduce", ADD, replica_groups=rg,
                        ins=[ar_in.opt()], outs=[ar_out.opt()])
                ss_all = qwp.tile([1, B], F32, tag="ss_all")
                nc.sync.dma_start(ss_all[:, :], ar_out[:, :])
                # rinv = exp(-0.5*ln(ms+eps)); ln/exp/copy share one ACT table
                lnv = qwp.tile([1, B], F32, tag="lnv")
                nc.scalar.activation(lnv[:, :], ss_all[:, :], LN,
                                     bias=eps_t[:1, :1], scale=1.0 / QL)
                rinv = qwp.tile([1, B], F32, tag="rinv")
                nc.scalar.activation(rinv[:, :], lnv[:, :], EXP, scale=-0.5)
                ps_bc = psqa.tile([128, B], F32, tag="bc")
                nc.tensor.matmul(ps_bc[:, :], ones_r_f[:1, :], rinv[:, :],
                                 start=True, stop=True)
                bc_sb = qwp.tile([128, B], F32, tag="bc_sb")
                nc.vector.tensor_copy(bc_sb[:, :], ps_bc[:, :])

                # normalized q_a^T (norm weight already folded into W2)
                qanT0 = qwp.tile([128, B], BF16, tag="qanT0")
                nc.vector.tensor_tensor(qanT0[:, :], ps_qa0[:, :],
                                        bc_sb[:, :], MULT)
                qanT1 = qwp.tile([QLP - 128, B], BF16, tag="qanT1")
                nc.vector.tensor_tensor(qanT1[:, :], ps_qa1[:, :],
                                        bc_sb[:QLP - 128, :], MULT)

            with (
                tc.tile_pool(name="psqr", bufs=3, space="PSUM") as psqr,
                tc.tile_pool(name="psqt", bufs=1, space="PSUM") as psqt,
            ):
                # W2 row matmul -> partial q_abs rows [32, 9216]
                qrows = qwp.tile([B, NQ], BF16, tag="qrows")
                for nchi in range(NCH):
                    n0 = nchi * 512
                    ps_r = psqr.tile([B, 512], F32, tag="qr")
                    nc.tensor.matmul(ps_r[:, :], qanT0[:, :],
                                     w2a_sb[:, n0:n0 + 512],
                                     start=True, stop=False)
                    nc.tensor.matmul(ps_r[:, :], qanT1[:, :],
                                     w2b_sb[:, n0:n0 + 512],
                                     start=False, stop=True)
                    if nchi % 2 == 0:
                        nc.vector.tensor_copy(qrows[:, n0:n0 + 512],
                                              ps_r[:, :])
                    else:
                        nc.scalar.copy(qrows[:, n0:n0 + 512], ps_r[:, :])

                # ReduceScatter: sum partials, keep my 4 sequences
                rs_in = dramp.tile([B, NQ], BF16)
                rs_out = dramp.tile([BP, NQ], BF16)
                nc.sync.dma_start(rs_in[:, :], qrows[:, :])
                if fake_coll:
                    nc.sync.dma_start(rs_out[:, :], rs_in[0:BP, :])
                else:
                    nc.gpsimd.collective_compute(
                        "ReduceScatter", ADD, replica_groups=rg,
                        ins=[rs_in.opt()], outs=[rs_out.opt()])
                qr4 = qwp.tile([BP, NQ], BF16, tag="qr4")
                nc.sync.dma_start(qr4[:, :], rs_out[:, :])

                # tiny transposes -> qabsT [c|128, (c h), b], qpeT [r, h, b]
                ps_qt = psqt.tile([128, H * 4 * BP], BF16, tag="qt")
                for g in range(H * 4):
                    nc.tensor.transpose(ps_qt[:, g * BP:(g + 1) * BP],
                                        qr4[:BP, g * 128:(g + 1) * 128],
                                        ident[:BP, :BP])
                nc.vector.tensor_copy(
                    qabsT[:, :, :],
                    ps_qt[:, :].rearrange("p (g b) -> p g b", b=BP))
                ps_qp = psqt.tile([DR, H * BP], BF16, tag="qp")
                for h in range(H):
                    nc.tensor.transpose(
                        ps_qp[:, h * BP:(h + 1) * BP],
                        qr4[:BP, NOPE + h * DR:NOPE + (h + 1) * DR],
                        ident[:BP, :BP])
                nc.vector.tensor_copy(
                    qpeT[:, :, :],
                    ps_qp[:, :].rearrange("p (h b) -> p h b", b=BP))
                if debug:
                    nc.sync.dma_start(dbg_qr4[:, :], qr4[:, :])

            # ================= attention =================
            ctxT = atp.tile([128, 4, H, BP], BF16)
            wvc_sb = osb.tile([128, H * 4, DV], BF16)
            wo_sb = osb.tile([128, H, HO], BF16)

            with (
                tc.tile_pool(name="pssc", bufs=1, space="PSUM") as pssc,
                tc.tile_pool(name="pstr", bufs=2, space="PSUM") as pstr,
                tc.tile_pool(name="psmi", bufs=1, space="PSUM") as psmi,
            ):
                mct = 0  # natc copy rotation counter

                for lb in range(BP):
                    if lb not in ctr_tiles:
                        issue_ctr(lb)
                    ps_sc = pssc.tile([128, 512], F32, tag="sc",
                                      name=f"sc{lb}")
                    # one accumulation chain per PSUM bank: interleaved
                    # starts within a bank abort each other's open group
                    ctx_c = [psmi.tile([128, H], F32, tag=f"ctx{c}",
                                       name=f"ctx{c}_{lb}")
                             for c in range(4)]
                    ps_sr = psmi.tile([128, 32], F32, tag="sr",
                                      name=f"sr{lb}")
                    sums_ap = ps_sr[:1, 0:16]
                    rb_ap = ps_sr[:, 16:32]
                    eT = etp.tile([128, ST * 4, H], BF16, tag="eT")
                    ctr_sb = ctr_tiles[lb]
                    prev = None  # deferred (st, natc tiles) for ctx stage

                    def ctx_stage(stage, lb=lb, eT=eT, ctx_c=ctx_c,
                                  sums_ap=sums_ap):
                        st, nats = stage
                        for i in range(4):
                            g = st * 4 + i
                            for c in range(4):
                                nc.tensor.matmul(
                                    ctx_c[c][:, :],
                                    nats[i][:, c * 128:(c + 1) * 128],
                                    eT[:, g, :],
                                    start=(g == 0),
                                    stop=(st == ST - 1 and i == 3))
                            nc.tensor.matmul(
                                sums_ap[:, :], ones_c_bf[:, :1], eT[:, g, :],
                                start=(g == 0), stop=(g == ST * 4 - 1))

                    for st in range(ST):
                        if (lb, st) not in ctl_tiles:
                            issue_ctl(lb, st)
                        ctl = ctl_tiles.pop((lb, st))
                        # scores [s, 16h]: cache tile stationary, q moving
                        for i in range(4):
                            sc_ap = ps_sc[:, (st * 4 + i) * 16:
                                          (st * 4 + i + 1) * 16]
                            for c in range(4):
                                nc.tensor.matmul(
                                    sc_ap[:, :],
                                    ctl[:, c, i * 128:(i + 1) * 128],
                                    qabsT[:, c * 16:(c + 1) * 16, lb],
                                    start=(c == 0), stop=False)
                            nc.tensor.matmul(
                                sc_ap[:, :],
                                ctr_sb[:, st * 512 + i * 128:
                                       st * 512 + (i + 1) * 128],
                                qpeT[:, :, lb], start=False, stop=True)
                        # exp straight out of PSUM into bf16 e^T
                        nc.scalar.activation(
                            eT[:, st * 4:(st + 1) * 4, :],
                            ps_sc[:, st * 64:(st + 1) * 64]
                            .rearrange("p (i h) -> p i h", i=4),
                            EXP, scale=SCALE)
                        # natural tiles via PE transpose + spread copies
                        nats = []
                        for i in range(4):
                            ps_tr = pstr.tile([128, 512], BF16, tag="tr")
                            for c in range(4):
                                nc.tensor.transpose(
                                    ps_tr[:, c * 128:(c + 1) * 128],
                                    ctl[:, c, i * 128:(i + 1) * 128],
                                    ident[:, :])
                            natc = natp.tile([128, 512], BF16, tag="nat")
                            r = mct % ncpy
                            mct += 1
                            if r < CPY[0]:
                                nc.vector.tensor_copy(natc[:, :], ps_tr[:, :])
                            else:
                                nc.scalar.copy(natc[:, :], ps_tr[:, :])
                            if debug and lb == 0 and st in (0, 7):
                                nc.sync.dma_start(
                                    dbg_nat[(st // 7) * 4 + i, :, :],
                                    natc[:, :])
                            nats.append(natc)
                        # context for the PREVIOUS stage (software pipeline
                        # so exp/copies have a stage of slack)
                        if prev is not None:
                            ctx_stage(prev)
                        prev = (st, nats)
                    ctx_stage(prev)

                    # softmax denom -> normalize into ctxT (bf16)
                    rec = atp.tile([1, H], F32, tag="rec", name=f"rec{lb}")
                    nc.vector.reciprocal(rec[:, :], sums_ap[:, :])
                    nc.tensor.matmul(rb_ap[:, :], ones_r_f[:1, :],
                                     rec[:, :], start=True, stop=True)
                    rb_sb = atp.tile([128, H], F32, tag="rb",
                                     name=f"rb{lb}")
                    nc.vector.tensor_copy(rb_sb[:, :], rb_ap[:, :])
                    for c in range(4):
                        nc.vector.tensor_tensor(
                            ctxT[:, c, :, lb], ctx_c[c][:, :], rb_sb[:, :],
                            MULT)

                    if lb == 1:
                        nc.sync.dma_start(wvc_sb[:, :, :], wvc[:, :, :])
                    if lb == 2:
                        nc.sync.dma_start(wo_sb[:, :, :], wo[:, :, :])
                    if debug and lb == 0:
                        nc.sync.dma_start(dbg_et[:, :, :], eT[:, :, :])

            # ================= output =================
            with (
                tc.tile_pool(name="psov", bufs=1, space="PSUM") as psov,
                tc.tile_pool(name="psoo", bufs=2, space="PSUM") as psoo,
            ):
                # un-absorb values: ovT [v, h, b]
                ps_ov = psov.tile([128, H, BP], F32, tag="ov")
                for h in range(H):
                    for c in range(4):
                        nc.tensor.matmul(ps_ov[:, h, :],
                                         wvc_sb[:, h * 4 + c, :],
                                         ctxT[:, c, h, :],
                                         start=(c == 0), stop=(c == 3))
                ovT_sb = osb.tile([128, H, BP], BF16)
                nc.vector.tensor_copy(ovT_sb[:, :, :], ps_ov[:, :, :])

                # -> rows [4, 2048] -> AllGather -> [32, 2048]
                ps_or0 = psov.tile([BP, 8 * DV], BF16, tag="or0")
                ps_or1 = psov.tile([BP, 8 * DV], BF16, tag="or1")
                for h in range(H):
                    pst = ps_or0 if h < 8 else ps_or1
                    nc.tensor.transpose(
                        pst[:, (h % 8) * DV:(h % 8 + 1) * DV],
                        ovT_sb[:, h, :], ident[:, :])
                ovr = osb.tile([BP, H * DV], BF16)
                nc.vector.tensor_copy(ovr[:, 0:8 * DV], ps_or0[:, :])
                nc.vector.tensor_copy(ovr[:, 8 * DV:], ps_or1[:, :])
                if debug:
                    nc.sync.dma_start(dbg_ctxT[:, :, :, :], ctxT[:, :, :, :])
                    nc.sync.dma_start(dbg_ovr[:, :], ovr[:, :])
                ag_in = dramp.tile([BP, H * DV], BF16)
                ag_out = dramp.tile([B, H * DV], BF16)
                nc.sync.dma_start(ag_in[:, :], ovr[:, :])
                if fake_coll:
                    nc.sync.dma_start(ag_out[0:BP, :], ag_in[:, :])
                else:
                    nc.gpsimd.collective_compute(
                        "AllGather", BYPASS, replica_groups=rg,
                        ins=[ag_in.opt()], outs=[ag_out.opt()])
                ov32 = osb.tile([B, H * DV], BF16)
                nc.sync.dma_start(ov32[:, :], ag_out[:, :])
                ps_ot = psov.tile([128, H * B], BF16, tag="ot")
                for kt in range(H):
                    nc.tensor.transpose(ps_ot[:, kt * B:(kt + 1) * B],
                                        ov32[:B, kt * 128:(kt + 1) * 128],
                                        ident[:B, :B])
                ovT_all = osb.tile([128, H, B], BF16)
                nc.vector.tensor_copy(
                    ovT_all[:, :, :],
                    ps_ot[:, :].rearrange("p (k b) -> p k b", b=B))

                # output projection (columns n0..n0+640 of the full output)
                outT_sb = osb.tile([128, 5, B], F32)
                for n in range(5):
                    ps_o = psoo.tile([128, B], F32, tag="oo")
                    for kt in range(H):
                        nc.tensor.matmul(ps_o[:, :],
                                         wo_sb[:, kt, n * 128:(n + 1) * 128],
                                         ovT_all[:, kt, :],
                                         start=(kt == 0), stop=(kt == H - 1))
                    nc.vector.tensor_copy(outT_sb[:, n, :], ps_o[:, :])
                nc.sync.dma_start(
                    out[:, :].rearrange("(n p) b -> p n b", p=128),
                    outT_sb[:, :, :])

    nc.compile()
    return nc


# ----------------------------- host wrapper ------------------------------


def _prep_in_maps(inputs, S, n_cores):
    hidden = np.asarray(inputs["hidden_states"], np.float32)
    pos = np.asarray(inputs["positions"], np.int32)
    w_qkv_a = np.asarray(inputs["w_qkv_a"], np.float32)
    q_a_norm_w = np.asarray(inputs["q_a_norm_w"], np.float32)
    w_q_b = np.asarray(inputs["w_q_b"], np.float32)
    kv_a_norm_w = np.asarray(inputs["kv_a_norm_w"], np.float32)
    w_kc = np.asarray(inputs["w_kc"], np.float32)
    w_vc = np.asarray(inputs["w_vc"], np.float32)
    w_o = np.asarray(inputs["w_o"], np.float32)
    cache_l = np.asarray(inputs["kv_cache_latent"], np.float32)
    cache_r = np.asarray(inputs["kv_cache_rope"], np.float32)

    # current-token cache update (host)
    latent = hidden @ w_qkv_a[:, QL:QL + KL]
    k_pe = hidden @ w_qkv_a[:, QL + KL:]
    cache_l = cache_l.copy()
    cache_r = cache_r.copy()
    cache_l[:, -1, :] = _rmsnorm_np(latent, kv_a_norm_w)
    cache_r[:, -1, :] = _rope_np(k_pe.astype(np.float32), pos)

    # fold q-rope into the rope cache: k' = R(pos_b)^T k
    inv = (1.0 / (BASE ** (np.arange(0, DR, 2, dtype=np.float32) / DR)))
    fr = pos.astype(np.float32)[:, None] * inv.astype(np.float32)
    cos = np.cos(fr).astype(np.float32)[:, None, :]
    sin = np.sin(fr).astype(np.float32)[:, None, :]
    cr1, cr2 = cache_r[..., 0::2], cache_r[..., 1::2]
    cr_rot = np.empty_like(cache_r)
    cr_rot[..., 0::2] = cos * cr1 + sin * cr2
    cr_rot[..., 1::2] = -sin * cr1 + cos * cr2

    # packed transposed caches, bf16
    # ctl[b] = [p, t, s] with c = t*128+p ; ctr[b] = [r, s]
    ctl_all = np.ascontiguousarray(
        cache_l[:, :S, :].transpose(0, 2, 1).reshape(B, 4, 128, S)
        .transpose(0, 2, 1, 3)).astype(BF)
    ctr_all = np.ascontiguousarray(cr_rot[:, :S, :].transpose(0, 2, 1)
                                   ).astype(BF)

    # W2 = [absorbed nope (c-chunk, h, 128) | rope (h, r)], norm scale folded
    w_qb_eff = q_a_norm_w[:, None] * w_q_b
    wq = w_qb_eff.reshape(QL, H, DN + DR)
    w_nope, w_pe = wq[:, :, :DN], wq[:, :, DN:]
    Wabs = np.einsum("qhd,hdc->qhc", w_nope, w_kc, optimize=True)
    nope_cols = Wabs.reshape(QL, H, 4, 128).transpose(0, 2, 1, 3).reshape(
        QL, NOPE)
    W2 = np.concatenate([nope_cols, w_pe.reshape(QL, H * DR)], axis=1)

    hT_p = np.ascontiguousarray(
        hidden.T.reshape(KTH, 128, B).transpose(1, 0, 2)).astype(BF)
    wvc_p = np.ascontiguousarray(
        w_vc.reshape(H, 4, 128, DV).transpose(2, 0, 1, 3).reshape(
            128, H * 4, DV)).astype(BF)

    in_maps = []
    for k in range(n_cores):
        b0 = k * BP
        q0 = k * QLP
        wqa_p = np.ascontiguousarray(
            w_qkv_a[:, q0:q0 + QLP].reshape(KTH, 128, QLP)
            .transpose(1, 0, 2)).astype(BF)
        w2s = W2[q0:q0 + QLP, :]
        wo_p = np.ascontiguousarray(
            w_o[:, k * HO:(k + 1) * HO].reshape(H, 128, HO)
            .transpose(1, 0, 2)).astype(BF)
        m = {
            "hT": hT_p,
            "wqa": wqa_p,
            "w2a": np.ascontiguousarray(w2s[0:128, :]).astype(BF),
            "w2b": np.ascontiguousarray(w2s[128:QLP, :]).astype(BF),
            "ctl": np.ascontiguousarray(ctl_all[b0:b0 + BP]),
            "ctr": np.ascontiguousarray(ctr_all[b0:b0 + BP]),
            "wvc": wvc_p,
            "wo": wo_p,
        }
        in_maps.append(m)
    return in_maps


def _unshard(results):
    return np.concatenate(
        [np.asarray(results[k]["out"], np.float32).T
         for k in range(N_CORES)], axis=1)


def run(inputs, S=4096, trace=False):
    key = (S, N_CORES)
    if key not in _CACHE:
        _CACHE[key] = _build(S, N_CORES)
    nc = _CACHE[key]
    in_maps = _prep_in_maps(inputs, S, N_CORES)
    res = bass_utils.run_bass_kernel_spmd(
        nc, in_maps, core_ids=list(range(N_CORES)), trace=trace)
    return _unshard(res.results), res


def kernel(**inputs) -> np.ndarray:
    out, _ = run(inputs)
    return out.astype(np.float32)


# revision 32
# speedup vs baseline: 2.7510x; 1.1291x over previous
"""DeepseekV2 MLA decode attention on 8 Trainium2 NeuronCores.

bf16 redesign. Strategy (single SPMD launch; per-core variation comes only
from in_maps contents and collective semantics):

  - Attention is batch-sharded: core k owns sequences 4k..4k+4. The latent KV
    cache is host-packed in ONE transposed bf16 layout [c, s]; the natural
    [s, c] tiles the context matmul needs are produced on-chip by PE
    transposes (bf16: 1 cycle/row) whose PSUM->SBUF copies are spread across
    DVE / ACT / GpSimd.  This reads the 16.8 MB/core cache exactly once.
  - Scores are computed as [s, 16h] PSUM tiles (cache tile stationary,
    absorbed-q moving, 16 columns/matmul); exp runs on ACT straight out of
    PSUM into a bf16 e^T tile; context is computed transposed ([c, h] out,
    natural tile stationary, e^T moving) so only 16 columns stream per
    matmul and the result lands directly in the layout w_vc consumes.
  - Rope is folded into the cache on the host: the rope-cache is pre-rotated
    by R(pos_b)^T per sequence, so the device does no rope at all.
  - q path: w_qkv_a's q columns are column-sharded (each core computes
    q_a[:, its 192 cols] for all 32 seqs); rmsnorm needs only an AllReduce of
    the 32 per-row sums of squares (128 B); w_kc and the q_a norm scale are
    folded into w_q_b on the host giving W2 [1536, 9216], K-sharded 192
    rows/core; the row-major partial q_abs [32, 9216] is ReduceScattered,
    which both sums partials and hands each core its 4 sequences.
  - w_o is column-sharded; ov rows are AllGathered (bf16) and each core
    produces a 640-column slice, concatenated on the host.
"""

import sys

sys.path.insert(0, "/opt/trn_rl_repo")

import ml_dtypes
import numpy as np

import concourse.bacc as bacc
import concourse.mybir as mybir
import concourse.tile as tile
from concourse import bass_utils
from concourse.masks import make_identity

F32 = mybir.dt.float32
BF16 = mybir.dt.bfloat16
ADD = mybir.AluOpType.add
MULT = mybir.AluOpType.mult
BYPASS = mybir.AluOpType.bypass
EXP = mybir.ActivationFunctionType.Exp
LN = mybir.ActivationFunctionType.Ln
SQUARE = mybir.ActivationFunctionType.Square

B, HID, H = 32, 5120, 16
DN, DR, DV = 128, 64, 128
QL, KL = 1536, 512
BASE = 10000.0
EPS = 1e-6
SCALE = float((DN + DR) ** -0.5)

N_CORES = 8
BP = B // N_CORES            # sequences per core
QLP = QL // N_CORES          # W2 contraction rows per core (192)
KTH = HID // 128             # hidden k-tiles (40)
NOPE = H * KL                # 8192 absorbed-nope columns of W2
NQ = NOPE + H * DR           # 9216 total W2 columns
HO = HID // N_CORES          # output columns per core (640)
NCH = NQ // 512              # n-chunks of the W2 row matmul (18)
TP = True                    # kept for test.py signature compat

BF = ml_dtypes.bfloat16

_CACHE = {}


# ----------------------------- host math ---------------------------------


def _rmsnorm_np(x, w):
    ms = np.mean(x * x, axis=-1, keepdims=True, dtype=np.float32)
    return (x * (1.0 / np.sqrt(ms + EPS)) * w).astype(np.float32)


def _rope_np(x, pos):
    d = x.shape[-1]
    inv = (1.0 / (BASE ** (np.arange(0, d, 2, dtype=np.float32) / d))).astype(
        np.float32
    )
    fr = pos.astype(np.float32)[:, None] * inv
    cos, sin = np.cos(fr).astype(np.float32), np.sin(fr).astype(np.float32)
    out = np.empty_like(x)
    out[..., 0::2] = x[..., 0::2] * cos - x[..., 1::2] * sin
    out[..., 1::2] = x[..., 1::2] * cos + x[..., 0::2] * sin
    return out.astype(np.float32)


# ----------------------------- device program ----------------------------


def _build(S, n_cores, tp=True, fake_coll=False, PRE=8, CPY=(3, 2),
           debug=False):
    nc = bacc.Bacc("TRN2", target_bir_lowering=False, debug=False,
                   enable_asserts=False, num_devices=n_cores)
    ST = S // 512
    rg = [list(range(n_cores))]
    ncpy = sum(CPY)

    hT = nc.dram_tensor("hT", [128, KTH, B], BF16, kind="ExternalInput")
    wqa = nc.dram_tensor("wqa", [128, KTH, QLP], BF16, kind="ExternalInput")
    w2a = nc.dram_tensor("w2a", [128, NQ], BF16, kind="ExternalInput")
    w2b = nc.dram_tensor("w2b", [QLP - 128, NQ], BF16, kind="ExternalInput")
    ctl_d = nc.dram_tensor("ctl", [BP, 128, 4, S], BF16, kind="ExternalInput")
    ctr_d = nc.dram_tensor("ctr", [BP, DR, S], BF16, kind="ExternalInput")
    wvc = nc.dram_tensor("wvc", [128, H * 4, DV], BF16, kind="ExternalInput")
    wo = nc.dram_tensor("wo", [128, H, HO], BF16, kind="ExternalInput")
    out = nc.dram_tensor("out", [HO, B], F32, kind="ExternalOutput")
    if debug:
        dbg_qr4 = nc.dram_tensor("dbg_qr4", [BP, NQ], BF16,
                                 kind="ExternalOutput")
        dbg_ctxT = nc.dram_tensor("dbg_ctxT", [128, 4, H, BP], BF16,
                                  kind="ExternalOutput")
        dbg_et = nc.dram_tensor("dbg_et", [128, S // 128, H], BF16,
                                kind="ExternalOutput")
        dbg_ovr = nc.dram_tensor("dbg_ovr", [BP, H * DV], BF16,
                                 kind="ExternalOutput")
        dbg_nat = nc.dram_tensor("dbg_nat", [8, 128, 512], BF16,
                                 kind="ExternalOutput")

    with tile.TileContext(nc) as tc:
        with (
            tc.tile_pool(name="const", bufs=1) as cp,
            tc.tile_pool(name="wq", bufs=1) as wqp,
            tc.tile_pool(name="qwork", bufs=1) as qwp,
            tc.tile_pool(name="dram", bufs=1, space="DRAM") as dramp,
            tc.tile_pool(name="ctl", bufs=PRE) as ctlp,
            tc.tile_pool(name="ctr", bufs=2) as ctrp,
            tc.tile_pool(name="natc", bufs=8) as natp,
            tc.tile_pool(name="et", bufs=2) as etp,
            tc.tile_pool(name="attn", bufs=1) as atp,
            tc.tile_pool(name="osb", bufs=1) as osb,
        ):
            # ---- constants ----
            ident = cp.tile([128, 128], BF16)
            make_identity(nc, ident[:, :])
            ones_c_bf = cp.tile([128, 1], BF16)
            nc.any.memset(ones_c_bf, 1.0)
            ones_c_f = cp.tile([128, 1], F32)
            nc.any.memset(ones_c_f, 1.0)
            ones_r_f = cp.tile([1, 128], F32)
            nc.any.memset(ones_r_f, 1.0)
            eps_t = cp.tile([1, 1], F32)
            nc.any.memset(eps_t, EPS)

            # ---- q-path weights; W2 streamed in n-column slices so its
            # matmul pipelines with its own load ----
            hT_sb = wqp.tile([128, KTH, B], BF16)
            nc.sync.dma_start(hT_sb[:, :, :], hT[:, :, :])
            wqa_sb = wqp.tile([128, KTH, QLP], BF16)
            nc.sync.dma_start(wqa_sb[:, :, :], wqa[:, :, :])
            w2a_sb = wqp.tile([128, NQ], BF16)
            w2b_sb = wqp.tile([QLP - 128, NQ], BF16)
            W2CH = 2048
            for n0 in range(0, NQ, W2CH):
                n1 = min(NQ, n0 + W2CH)
                nc.sync.dma_start(w2a_sb[:, n0:n1], w2a[:, n0:n1])
                nc.sync.dma_start(w2b_sb[:, n0:n1], w2b[:, n0:n1])

            # ---- cache prefetch (covers the ReduceScatter latency gap) ----
            ctl_tiles = {}

            def issue_ctl(lb, st):
                t = ctlp.tile([128, 4, 512], BF16, tag="ctl")
                nc.sync.dma_start(t[:, :, :],
                                  ctl_d[lb, :, :, st * 512:(st + 1) * 512])
                ctl_tiles[(lb, st)] = t

            ctr_tiles = {}

            def issue_ctr(lb):
                t = ctrp.tile([DR, S], BF16, tag="ctr")
                nc.sync.dma_start(t[:, :], ctr_d[lb, :, :])
                ctr_tiles[lb] = t

            issue_ctr(0)
            for m in range(3):
                issue_ctl(0, m)

            # ================= q path =================
            qabsT = qwp.tile([128, H * 4, BP], BF16)
            qpeT = qwp.tile([DR, H, BP], BF16)

            NQX = NQ + 1  # extra column carries the sum-of-squares row
            with (
                tc.tile_pool(name="psqa", bufs=1, space="PSUM") as psqa,
                tc.tile_pool(name="psqr", bufs=3, space="PSUM") as psqr,
            ):
                # q_a^T (unnormalized) for my 192 columns, all 32 seqs
                ps_qa0 = psqa.tile([128, B], F32, tag="qa0")
                ps_qa1 = psqa.tile([QLP - 128, B], F32, tag="qa1")
                for kt in range(KTH):
                    nc.tensor.matmul(ps_qa0[:, :], wqa_sb[:, kt, 0:128],
                                     hT_sb[:, kt, :],
                                     start=(kt == 0), stop=(kt == KTH - 1))
                    nc.tensor.matmul(ps_qa1[:, :], wqa_sb[:, kt, 128:QLP],
                                     hT_sb[:, kt, :],
                                     start=(kt == 0), stop=(kt == KTH - 1))
                qaT0 = qwp.tile([128, B], BF16, tag="qaT0")
                nc.vector.tensor_copy(qaT0[:, :], ps_qa0[:, :])
                qaT1 = qwp.tile([QLP - 128, B], BF16, tag="qaT1")
                nc.vector.tensor_copy(qaT1[:, :], ps_qa1[:, :])

                # partial mean-of-squares as a ROWS column -> rides the RS,
                # which hands every core exactly its 4 sequences' sums
                sq0 = qwp.tile([128, B], BF16, tag="sq0")
                nc.scalar.activation(sq0[:, :], ps_qa0[:, :], SQUARE,
                                     scale=float(QL) ** -0.5)
                sq1 = qwp.tile([QLP - 128, B], BF16, tag="sq1")
                nc.scalar.activation(sq1[:, :], ps_qa1[:, :], SQUARE,
                                     scale=float(QL) ** -0.5)
                ps_ssr = psqa.tile([B, 1], F32, tag="ssr")
                nc.tensor.matmul(ps_ssr[:, :], sq0[:, :], ones_c_bf[:, :],
                                 start=True, stop=False)
                nc.tensor.matmul(ps_ssr[:, :], sq1[:, :],
                                 ones_c_bf[:QLP - 128, :],
                                 start=False, stop=True)

                # W2 row matmul -> partial q_abs rows [32, 9216 + 1]
                qrows = qwp.tile([B, NQX], BF16, tag="qrows")
                for nchi in range(NCH):
                    n0 = nchi * 512
                    ps_r = psqr.tile([B, 512], F32, tag="qr")
                    nc.tensor.matmul(ps_r[:, :], qaT0[:, :],
                                     w2a_sb[:, n0:n0 + 512],
                                     start=True, stop=False)
                    nc.tensor.matmul(ps_r[:, :], qaT1[:, :],
                                     w2b_sb[:, n0:n0 + 512],
                                     start=False, stop=True)
                    if nchi % 2 == 0:
                        nc.vector.tensor_copy(qrows[:, n0:n0 + 512],
                                              ps_r[:, :])
                    else:
                        nc.scalar.copy(qrows[:, n0:n0 + 512], ps_r[:, :])
                nc.vector.tensor_copy(qrows[:, NQ:NQX], ps_ssr[:, :])

                # ReduceScatter: sum partials, keep my 4 sequences
                rs_in = dramp.tile([B, NQX], BF16)
                rs_out = dramp.tile([BP, NQX], BF16)
                nc.sync.dma_start(rs_in[:, :], qrows[:, :])
                if fake_coll:
                    nc.sync.dma_start(rs_out[:, :], rs_in[0:BP, :])
                else:
                    nc.gpsimd.collective_compute(
                        "ReduceScatter", ADD, replica_groups=rg,
                        ins=[rs_in.opt()], outs=[rs_out.opt()])

            with tc.tile_pool(name="psqt", bufs=1, space="PSUM") as psqt:
                qr4 = qwp.tile([BP, NQX], BF16, tag="qr4")
                nc.sync.dma_start(qr4[:, :], rs_out[:, :])

                # rinv for my 4 seqs: exp(-0.5*ln(ms+eps)), broadcast to a
                # [128, 4] column tile (ln/exp/square/copy share one table)
                ps_sst = psqt.tile([1, BP], BF16, tag="sst")
                nc.tensor.transpose(ps_sst[:, :], qr4[:BP, NQ:NQX],
                                    ident[:BP, :BP])
                lnv = qwp.tile([1, BP], F32, tag="lnv")
                nc.scalar.activation(lnv[:, :], ps_sst[:, :], LN,
                                     bias=eps_t[:1, :1])
                rinv = qwp.tile([1, BP], F32, tag="rinv")
                nc.scalar.activation(rinv[:, :], lnv[:, :], EXP, scale=-0.5)
                ps_bc4 = psqt.tile([128, BP], F32, tag="bc4")
                nc.tensor.matmul(ps_bc4[:, :], ones_r_f[:1, :], rinv[:, :],
                                 start=True, stop=True)
                bc4 = qwp.tile([128, BP], BF16, tag="bc4s")
                nc.vector.tensor_copy(bc4[:, :], ps_bc4[:, :])

                # tiny transposes -> qabsT [c|128, (c h), b], qpeT [r, h, b]
                # (rinv applied during the PSUM->SBUF move)
                ps_qt = psqt.tile([128, H * 4 * BP], BF16, tag="qt")
                for g in range(H * 4):
                    nc.tensor.transpose(ps_qt[:, g * BP:(g + 1) * BP],
                                        qr4[:BP, g * 128:(g + 1) * 128],
                                        ident[:BP, :BP])
                nc.vector.tensor_tensor(
                    qabsT[:, :, :],
                    ps_qt[:, :].rearrange("p (g b) -> p g b", b=BP),
                    bc4[:, :].rearrange("p (o b) -> p o b", o=1)
                    .broadcast_to([128, H * 4, BP]), MULT)
                ps_qp = psqt.tile([DR, H * BP], BF16, tag="qp")
                for h in range(H):
                    nc.tensor.transpose(
                        ps_qp[:, h * BP:(h + 1) * BP],
                        qr4[:BP, NOPE + h * DR:NOPE + (h + 1) * DR],
                        ident[:BP, :BP])
                nc.vector.tensor_tensor(
                    qpeT[:, :, :],
                    ps_qp[:, :].rearrange("p (h b) -> p h b", b=BP),
                    bc4[:DR, :].rearrange("p (o b) -> p o b", o=1)
                    .broadcast_to([DR, H, BP]), MULT)
                if debug:
                    nc.sync.dma_start(dbg_qr4[:, :], qr4[:, 0:NQ])

            # ================= attention =================
            ctxT = atp.tile([128, 4, H, BP], BF16)
            wvc_sb = osb.tile([128, H * 4, DV], BF16)
            wo_sb = osb.tile([128, H, HO], BF16)

            with (
                tc.tile_pool(name="pssc", bufs=1, space="PSUM") as pssc,
                tc.tile_pool(name="pstr", bufs=2, space="PSUM") as pstr,
                tc.tile_pool(name="psmi", bufs=1, space="PSUM") as psmi,
            ):
                mct = 0  # natc copy rotation counter
                # one shared sums bank: per-seq chains in disjoint regions
                # open strictly one after another (complete before next opens)
                ps_sums = psmi.tile([1, BP * H], F32, tag="sums")

                for lb in range(BP):
                    if lb not in ctr_tiles:
                        issue_ctr(lb)
                    ps_sc = pssc.tile([128, 512], F32, tag="sc",
                                      name=f"sc{lb}")
                    # one accumulation chain per PSUM bank: interleaved
                    # starts within a bank abort each other's open group
                    ctx_c = [psmi.tile([128, H], F32, tag=f"ctx{c}",
                                       name=f"ctx{c}_{lb}")
                             for c in range(4)]
                    sums_ap = ps_sums[:1, lb * H:(lb + 1) * H]
                    eT = etp.tile([128, ST * 4, H], BF16, tag="eT")
                    ctr_sb = ctr_tiles[lb]
                    prev = None  # deferred (st, natc tiles) for ctx stage

                    def ctx_stage(stage, lb=lb, eT=eT, ctx_c=ctx_c,
                                  sums_ap=sums_ap):
                        st, nats = stage
                        for i in range(4):
                            g = st * 4 + i
                            for c in range(4):
                                nc.tensor.matmul(
                                    ctx_c[c][:, :],
                                    nats[i][:, c * 128:(c + 1) * 128],
                                    eT[:, g, :],
                                    start=(g == 0),
                                    stop=(st == ST - 1 and i == 3))
                            nc.tensor.matmul(
                                sums_ap[:, :], ones_c_bf[:, :1], eT[:, g, :],
                                start=(g == 0), stop=(g == ST * 4 - 1))

                    for st in range(ST):
                        if (lb, st) not in ctl_tiles:
                            issue_ctl(lb, st)
                        ctl = ctl_tiles.pop((lb, st))
                        # scores [s, 16h]: cache tile stationary, q moving
                        for i in range(4):
                            sc_ap = ps_sc[:, (st * 4 + i) * 16:
                                          (st * 4 + i + 1) * 16]
                            for c in range(4):
                                nc.tensor.matmul(
                                    sc_ap[:, :],
                                    ctl[:, c, i * 128:(i + 1) * 128],
                                    qabsT[:, c * 16:(c + 1) * 16, lb],
                                    start=(c == 0), stop=False)
                            nc.tensor.matmul(
                                sc_ap[:, :],
                                ctr_sb[:, st * 512 + i * 128:
                                       st * 512 + (i + 1) * 128],
                                qpeT[:, :, lb], start=False, stop=True)
                        # exp straight out of PSUM into bf16 e^T
                        nc.scalar.activation(
                            eT[:, st * 4:(st + 1) * 4, :],
                            ps_sc[:, st * 64:(st + 1) * 64]
                            .rearrange("p (i h) -> p i h", i=4),
                            EXP, scale=SCALE)
                        # natural tiles via PE transpose + spread copies
                        nats = []
                        for i in range(4):
                            ps_tr = pstr.tile([128, 512], BF16, tag="tr")
                            for c in range(4):
                                nc.tensor.transpose(
                                    ps_tr[:, c * 128:(c + 1) * 128],
                                    ctl[:, c, i * 128:(i + 1) * 128],
                                    ident[:, :])
                            natc = natp.tile([128, 512], BF16, tag="nat")
                            r = mct % ncpy
                            mct += 1
                            if r < CPY[0]:
                                nc.vector.tensor_copy(natc[:, :], ps_tr[:, :])
                            else:
                                nc.scalar.copy(natc[:, :], ps_tr[:, :])
                            if debug and lb == 0 and st in (0, 7):
                                nc.sync.dma_start(
                                    dbg_nat[(st // 7) * 4 + i, :, :],
                                    natc[:, :])
                            nats.append(natc)
                        # context for the PREVIOUS stage (software pipeline
                        # so exp/copies have a stage of slack)
                        if prev is not None:
                            ctx_stage(prev)
                        prev = (st, nats)
                    ctx_stage(prev)

                    # unnormalized ctx -> ctxT (softmax denom applied at the
                    # ovT stage, once for all 4 seqs)
                    for c in range(4):
                        if c % 2 == 0:
                            nc.vector.tensor_copy(ctxT[:, c, :, lb],
                                                  ctx_c[c][:, :])
                        else:
                            nc.scalar.copy(ctxT[:, c, :, lb], ctx_c[c][:, :])

                    if lb == 1:
                        nc.sync.dma_start(wvc_sb[:, :, :], wvc[:, :, :])
                    if lb == 2:
                        nc.sync.dma_start(wo_sb[:, :, :], wo[:, :, :])
                    if debug and lb == 0:
                        nc.sync.dma_start(dbg_et[:, :, :], eT[:, :, :])

                # reciprocal of all 32 softmax denominators, (b, h) order
                recip_sb = atp.tile([1, BP * H], F32)
                nc.vector.reciprocal(recip_sb[:, :], ps_sums[:1, :])

            # ================= output =================
            with (
                tc.tile_pool(name="psov", bufs=1, space="PSUM") as psov,
                tc.tile_pool(name="psoo", bufs=2, space="PSUM") as psoo,
            ):
                # softmax denominators broadcast down the partitions
                ps_rba = psov.tile([128, H, BP], F32, tag="rba")
                nc.tensor.matmul(
                    ps_rba[:, :, :], ones_r_f[:1, :],
                    recip_sb[:, :].rearrange("p (b h) -> p h b", h=H),
                    start=True, stop=True)
                rb_all = osb.tile([128, H, BP], BF16)
                nc.vector.tensor_copy(rb_all[:, :, :], ps_rba[:, :, :])

                # un-absorb values: ovT [v, h, b], normalized here
                ps_ov = psov.tile([128, H, BP], F32, tag="ov")
                for h in range(H):
                    for c in range(4):
                        nc.tensor.matmul(ps_ov[:, h, :],
                                         wvc_sb[:, h * 4 + c, :],
                                         ctxT[:, c, h, :],
                                         start=(c == 0), stop=(c == 3))
                ovT_sb = osb.tile([128, H, BP], BF16)
                nc.vector.tensor_tensor(ovT_sb[:, :, :], ps_ov[:, :, :],
                                        rb_all[:, :, :], MULT)

                # -> rows [4, 2048] -> AllGather -> [32, 2048]
                ps_or0 = psov.tile([BP, 8 * DV], BF16, tag="or0")
                ps_or1 = psov.tile([BP, 8 * DV], BF16, tag="or1")
                for h in range(H):
                    pst = ps_or0 if h < 8 else ps_or1
                    nc.tensor.transpose(
                        pst[:, (h % 8) * DV:(h % 8 + 1) * DV],
                        ovT_sb[:, h, :], ident[:, :])
                ovr = osb.tile([BP, H * DV], BF16)
                nc.vector.tensor_copy(ovr[:, 0:8 * DV], ps_or0[:, :])
                nc.vector.tensor_copy(ovr[:, 8 * DV:], ps_or1[:, :])
                if debug:
                    nc.sync.dma_start(dbg_ctxT[:, :, :, :], ctxT[:, :, :, :])
                    nc.sync.dma_start(dbg_ovr[:, :], ovr[:, :])
                ag_in = dramp.tile([BP, H * DV], BF16)
                ag_out = dramp.tile([B, H * DV], BF16)
                nc.sync.dma_start(ag_in[:, :], ovr[:, :])
                if fake_coll:
                    nc.sync.dma_start(ag_out[0:BP, :], ag_in[:, :])
                else:
                    nc.gpsimd.collective_compute(
                        "AllGather", BYPASS, replica_groups=rg,
                        ins=[ag_in.opt()], outs=[ag_out.opt()])
                ov32 = osb.tile([B, H * DV], BF16)
                nc.sync.dma_start(ov32[:, :], ag_out[:, :])
                ps_ot = psov.tile([128, H * B], BF16, tag="ot")
                for kt in range(H):
                    nc.tensor.transpose(ps_ot[:, kt * B:(kt + 1) * B],
                                        ov32[:B, kt * 128:(kt + 1) * 128],
                                        ident[:B, :B])
                ovT_all = osb.tile([128, H, B], BF16)
                nc.vector.tensor_copy(
                    ovT_all[:, :, :],
                    ps_ot[:, :].rearrange("p (k b) -> p k b", b=B))

                # output projection (columns n0..n0+640 of the full output)
                outT_sb = osb.tile([128, 5, B], F32)
                for n in range(5):
                    ps_o = psoo.tile([128, B], F32, tag="oo")
                    for kt in range(H):
                        nc.tensor.matmul(ps_o[:, :],
                                         wo_sb[:, kt, n * 128:(n + 1) * 128],
                                         ovT_all[:, kt, :],
                                         start=(kt == 0), stop=(kt == H - 1))
                    nc.vector.tensor_copy(outT_sb[:, n, :], ps_o[:, :])
                nc.sync.dma_start(
                    out[:, :].rearrange("(n p) b -> p n b", p=128),
                    outT_sb[:, :, :])

    nc.compile()
    return nc


# ----------------------------- host wrapper ------------------------------


def _prep_in_maps(inputs, S, n_cores):
    hidden = np.asarray(inputs["hidden_states"], np.float32)
    pos = np.asarray(inputs["positions"], np.int32)
    w_qkv_a = np.asarray(inputs["w_qkv_a"], np.float32)
    q_a_norm_w = np.asarray(inputs["q_a_norm_w"], np.float32)
    w_q_b = np.asarray(inputs["w_q_b"], np.float32)
    kv_a_norm_w = np.asarray(inputs["kv_a_norm_w"], np.float32)
    w_kc = np.asarray(inputs["w_kc"], np.float32)
    w_vc = np.asarray(inputs["w_vc"], np.float32)
    w_o = np.asarray(inputs["w_o"], np.float32)
    cache_l = np.asarray(inputs["kv_cache_latent"], np.float32)
    cache_r = np.asarray(inputs["kv_cache_rope"], np.float32)

    # current-token cache update (host)
    latent = hidden @ w_qkv_a[:, QL:QL + KL]
    k_pe = hidden @ w_qkv_a[:, QL + KL:]
    cache_l = cache_l.copy()
    cache_r = cache_r.copy()
    cache_l[:, -1, :] = _rmsnorm_np(latent, kv_a_norm_w)
    cache_r[:, -1, :] = _rope_np(k_pe.astype(np.float32), pos)

    # fold q-rope into the rope cache: k' = R(pos_b)^T k
    inv = (1.0 / (BASE ** (np.arange(0, DR, 2, dtype=np.float32) / DR)))
    fr = pos.astype(np.float32)[:, None] * inv.astype(np.float32)
    cos = np.cos(fr).astype(np.float32)[:, None, :]
    sin = np.sin(fr).astype(np.float32)[:, None, :]
    cr1, cr2 = cache_r[..., 0::2], cache_r[..., 1::2]
    cr_rot = np.empty_like(cache_r)
    cr_rot[..., 0::2] = cos * cr1 + sin * cr2
    cr_rot[..., 1::2] = -sin * cr1 + cos * cr2

    # packed transposed caches, bf16
    # ctl[b] = [p, t, s] with c = t*128+p ; ctr[b] = [r, s]
    ctl_all = np.ascontiguousarray(
        cache_l[:, :S, :].transpose(0, 2, 1).reshape(B, 4, 128, S)
        .transpose(0, 2, 1, 3)).astype(BF)
    ctr_all = np.ascontiguousarray(cr_rot[:, :S, :].transpose(0, 2, 1)
                                   ).astype(BF)

    # W2 = [absorbed nope (c-chunk, h, 128) | rope (h, r)], norm scale folded
    w_qb_eff = q_a_norm_w[:, None] * w_q_b
    wq = w_qb_eff.reshape(QL, H, DN + DR)
    w_nope, w_pe = wq[:, :, :DN], wq[:, :, DN:]
    Wabs = np.einsum("qhd,hdc->qhc", w_nope, w_kc, optimize=True)
    nope_cols = Wabs.reshape(QL, H, 4, 128).transpose(0, 2, 1, 3).reshape(
        QL, NOPE)
    W2 = np.concatenate([nope_cols, w_pe.reshape(QL, H * DR)], axis=1)

    hT_p = np.ascontiguousarray(
        hidden.T.reshape(KTH, 128, B).transpose(1, 0, 2)).astype(BF)
    wvc_p = np.ascontiguousarray(
        w_vc.reshape(H, 4, 128, DV).transpose(2, 0, 1, 3).reshape(
            128, H * 4, DV)).astype(BF)

    in_maps = []
    for k in range(n_cores):
        b0 = k * BP
        q0 = k * QLP
        wqa_p = np.ascontiguousarray(
            w_qkv_a[:, q0:q0 + QLP].reshape(KTH, 128, QLP)
            .transpose(1, 0, 2)).astype(BF)
        w2s = W2[q0:q0 + QLP, :]
        wo_p = np.ascontiguousarray(
            w_o[:, k * HO:(k + 1) * HO].reshape(H, 128, HO)
            .transpose(1, 0, 2)).astype(BF)
        m = {
            "hT": hT_p,
            "wqa": wqa_p,
            "w2a": np.ascontiguousarray(w2s[0:128, :]).astype(BF),
            "w2b": np.ascontiguousarray(w2s[128:QLP, :]).astype(BF),
            "ctl": np.ascontiguousarray(ctl_all[b0:b0 + BP]),
            "ctr": np.ascontiguousarray(ctr_all[b0:b0 + BP]),
            "wvc": wvc_p,
            "wo": wo_p,
        }
        in_maps.append(m)
    return in_maps


def _unshard(results):
    return np.concatenate(
        [np.asarray(results[k]["out"], np.float32).T
         for k in range(N_CORES)], axis=1)


def run(inputs, S=4096, trace=False):
    key = (S, N_CORES)
    if key not in _CACHE:
        _CACHE[key] = _build(S, N_CORES)
    nc = _CACHE[key]
    in_maps = _prep_in_maps(inputs, S, N_CORES)
    res = bass_utils.run_bass_kernel_spmd(
        nc, in_maps, core_ids=list(range(N_CORES)), trace=trace)
    return _unshard(res.results), res


def kernel(**inputs) -> np.ndarray:
    out, _ = run(inputs)
    return out.astype(np.float32)


# revision 42
# speedup vs baseline: 2.7814x; 1.0111x over previous
"""DeepseekV2 MLA decode attention on 8 Trainium2 NeuronCores.

bf16 redesign. Strategy (single SPMD launch; per-core variation comes only
from in_maps contents and collective semantics):

  - Attention is batch-sharded: core k owns sequences 4k..4k+4. The latent KV
    cache is host-packed in ONE transposed bf16 layout [c, s]; the natural
    [s, c] tiles the context matmul needs are produced on-chip by PE
    transposes (bf16: 1 cycle/row) whose PSUM->SBUF copies are spread across
    DVE / ACT / GpSimd.  This reads the 16.8 MB/core cache exactly once.
  - Scores are computed as [s, 16h] PSUM tiles (cache tile stationary,
    absorbed-q moving, 16 columns/matmul); exp runs on ACT straight out of
    PSUM into a bf16 e^T tile; context is computed transposed ([c, h] out,
    natural tile stationary, e^T moving) so only 16 columns stream per
    matmul and the result lands directly in the layout w_vc consumes.
  - Rope is folded into the cache on the host: the rope-cache is pre-rotated
    by R(pos_b)^T per sequence, so the device does no rope at all.
  - q path: w_qkv_a's q columns are column-sharded (each core computes
    q_a[:, its 192 cols] for all 32 seqs); rmsnorm needs only an AllReduce of
    the 32 per-row sums of squares (128 B); w_kc and the q_a norm scale are
    folded into w_q_b on the host giving W2 [1536, 9216], K-sharded 192
    rows/core; the row-major partial q_abs [32, 9216] is ReduceScattered,
    which both sums partials and hands each core its 4 sequences.
  - w_o is column-sharded; ov rows are AllGathered (bf16) and each core
    produces a 640-column slice, concatenated on the host.
"""

import sys

sys.path.insert(0, "/opt/trn_rl_repo")

import ml_dtypes
import numpy as np

import concourse.bacc as bacc
import concourse.mybir as mybir
import concourse.tile as tile
from concourse import bass_utils
from concourse.masks import make_identity

F32 = mybir.dt.float32
BF16 = mybir.dt.bfloat16
ADD = mybir.AluOpType.add
MULT = mybir.AluOpType.mult
BYPASS = mybir.AluOpType.bypass
EXP = mybir.ActivationFunctionType.Exp
LN = mybir.ActivationFunctionType.Ln
SQUARE = mybir.ActivationFunctionType.Square

B, HID, H = 32, 5120, 16
DN, DR, DV = 128, 64, 128
QL, KL = 1536, 512
BASE = 10000.0
EPS = 1e-6
SCALE = float((DN + DR) ** -0.5)

N_CORES = 8
BP = B // N_CORES            # sequences per core
QLP = QL // N_CORES          # W2 contraction rows per core (192)
KTH = HID // 128             # hidden k-tiles (40)
NOPE = H * KL                # 8192 absorbed-nope columns of W2
NQ = NOPE + H * DR           # 9216 total W2 columns
HO = HID // N_CORES          # output columns per core (640)
NCH = NQ // 512              # n-chunks of the W2 row matmul (18)
TP = True                    # kept for test.py signature compat

BF = ml_dtypes.bfloat16

_CACHE = {}


# ----------------------------- host math ---------------------------------


def _rmsnorm_np(x, w):
    ms = np.mean(x * x, axis=-1, keepdims=True, dtype=np.float32)
    return (x * (1.0 / np.sqrt(ms + EPS)) * w).astype(np.float32)


def _rope_np(x, pos):
    d = x.shape[-1]
    inv = (1.0 / (BASE ** (np.arange(0, d, 2, dtype=np.float32) / d))).astype(
        np.float32
    )
    fr = pos.astype(np.float32)[:, None] * inv
    cos, sin = np.cos(fr).astype(np.float32), np.sin(fr).astype(np.float32)
    out = np.empty_like(x)
    out[..., 0::2] = x[..., 0::2] * cos - x[..., 1::2] * sin
    out[..., 1::2] = x[..., 1::2] * cos + x[..., 0::2] * sin
    return out.astype(np.float32)


# ----------------------------- device program ----------------------------


def _build(S, n_cores, tp=True, fake_coll=False, PRE=10, CPY=(2, 1),
           debug=False):
    nc = bacc.Bacc("TRN2", target_bir_lowering=False, debug=False,
                   enable_asserts=False, num_devices=n_cores)
    ST = S // 512
    rg = [list(range(n_cores))]
    ncpy = sum(CPY)

    hT = nc.dram_tensor("hT", [128, KTH, B], BF16, kind="ExternalInput")
    wqa = nc.dram_tensor("wqa", [128, KTH, QLP], BF16, kind="ExternalInput")
    w2a = nc.dram_tensor("w2a", [128, NQ], BF16, kind="ExternalInput")
    w2b = nc.dram_tensor("w2b", [QLP - 128, NQ], BF16, kind="ExternalInput")
    ctl_d = nc.dram_tensor("ctl", [BP, 128, 4, S], BF16, kind="ExternalInput")
    ctr_d = nc.dram_tensor("ctr", [BP, DR, S], BF16, kind="ExternalInput")
    cnat_d = nc.dram_tensor("cnat", [BP, S, KL], BF16, kind="ExternalInput")
    wvc = nc.dram_tensor("wvc", [128, H * 4, DV], BF16, kind="ExternalInput")
    wo = nc.dram_tensor("wo", [128, H, HO], BF16, kind="ExternalInput")
    out = nc.dram_tensor("out", [HO, B], F32, kind="ExternalOutput")
    if debug:
        dbg_qr4 = nc.dram_tensor("dbg_qr4", [BP, NQ], BF16,
                                 kind="ExternalOutput")
        dbg_ctxT = nc.dram_tensor("dbg_ctxT", [128, 4, H, BP], BF16,
                                  kind="ExternalOutput")
        dbg_et = nc.dram_tensor("dbg_et", [128, S // 128, H], BF16,
                                kind="ExternalOutput")
        dbg_ovr = nc.dram_tensor("dbg_ovr", [BP, H * DV], BF16,
                                 kind="ExternalOutput")

    with tile.TileContext(nc) as tc:
        with (
            tc.tile_pool(name="const", bufs=1) as cp,
            tc.tile_pool(name="wq", bufs=1) as wqp,
            tc.tile_pool(name="qwork", bufs=1) as qwp,
            tc.tile_pool(name="dram", bufs=1, space="DRAM") as dramp,
            tc.tile_pool(name="ctl", bufs=PRE) as ctlp,
            tc.tile_pool(name="ctr", bufs=2) as ctrp,
            tc.tile_pool(name="natc", bufs=14) as natp,
            tc.tile_pool(name="et", bufs=2) as etp,
            tc.tile_pool(name="attn", bufs=1) as atp,
            tc.tile_pool(name="osb", bufs=1) as osb,
        ):
            # ---- constants ----
            ident = cp.tile([128, 128], BF16)
            make_identity(nc, ident[:, :])
            ones_c_bf = cp.tile([128, 1], BF16)
            nc.any.memset(ones_c_bf, 1.0)
            ones_c_f = cp.tile([128, 1], F32)
            nc.any.memset(ones_c_f, 1.0)
            ones_r_f = cp.tile([1, 128], F32)
            nc.any.memset(ones_r_f, 1.0)
            eps_t = cp.tile([1, 1], F32)
            nc.any.memset(eps_t, EPS)

            # ---- q-path weights; W2 streamed in n-column slices so its
            # matmul pipelines with its own load ----
            hT_sb = wqp.tile([128, KTH, B], BF16)
            nc.sync.dma_start(hT_sb[:, :, :], hT[:, :, :])
            wqa_sb = wqp.tile([128, KTH, QLP], BF16)
            nc.sync.dma_start(wqa_sb[:, :, :], wqa[:, :, :])
            w2a_sb = wqp.tile([128, NQ], BF16)
            w2b_sb = wqp.tile([QLP - 128, NQ], BF16)
            W2CH = 2048
            for n0 in range(0, NQ, W2CH):
                n1 = min(NQ, n0 + W2CH)
                nc.sync.dma_start(w2a_sb[:, n0:n1], w2a[:, n0:n1])
                nc.sync.dma_start(w2b_sb[:, n0:n1], w2b[:, n0:n1])

            # ---- cache prefetch (covers the ReduceScatter latency gap) ----
            ctl_tiles = {}

            def issue_ctl(lb, st):
                t = ctlp.tile([128, 4, 512], BF16, tag="ctl")
                nc.sync.dma_start(t[:, :, :],
                                  ctl_d[lb, :, :, st * 512:(st + 1) * 512])
                ctl_tiles[(lb, st)] = t

            ctr_tiles = {}

            def issue_ctr(lb):
                t = ctrp.tile([DR, S], BF16, tag="ctr")
                nc.sync.dma_start(t[:, :], ctr_d[lb, :, :])
                ctr_tiles[lb] = t

            issue_ctr(0)
            for m in range(3):
                issue_ctl(0, m)

            # ================= q path =================
            qabsT = qwp.tile([128, H * 4, BP], BF16)
            qpeT = qwp.tile([DR, H, BP], BF16)

            NQX = NQ + 1  # extra column carries the sum-of-squares row
            with (
                tc.tile_pool(name="psqa", bufs=1, space="PSUM") as psqa,
                tc.tile_pool(name="psqr", bufs=3, space="PSUM") as psqr,
            ):
                # q_a^T (unnormalized) for my 192 columns, all 32 seqs
                ps_qa0 = psqa.tile([128, B], F32, tag="qa0")
                ps_qa1 = psqa.tile([QLP - 128, B], F32, tag="qa1")
                for kt in range(KTH):
                    nc.tensor.matmul(ps_qa0[:, :], wqa_sb[:, kt, 0:128],
                                     hT_sb[:, kt, :],
                                     start=(kt == 0), stop=(kt == KTH - 1))
                    nc.tensor.matmul(ps_qa1[:, :], wqa_sb[:, kt, 128:QLP],
                                     hT_sb[:, kt, :],
                                     start=(kt == 0), stop=(kt == KTH - 1))
                qaT0 = qwp.tile([128, B], BF16, tag="qaT0")
                nc.vector.tensor_copy(qaT0[:, :], ps_qa0[:, :])
                qaT1 = qwp.tile([QLP - 128, B], BF16, tag="qaT1")
                nc.vector.tensor_copy(qaT1[:, :], ps_qa1[:, :])

                # partial mean-of-squares as a ROWS column -> rides the RS,
                # which hands every core exactly its 4 sequences' sums
                sq0 = qwp.tile([128, B], BF16, tag="sq0")
                nc.scalar.activation(sq0[:, :], ps_qa0[:, :], SQUARE,
                                     scale=float(QL) ** -0.5)
                sq1 = qwp.tile([QLP - 128, B], BF16, tag="sq1")
                nc.scalar.activation(sq1[:, :], ps_qa1[:, :], SQUARE,
                                     scale=float(QL) ** -0.5)
                ps_ssr = psqa.tile([B, 1], F32, tag="ssr")
                nc.tensor.matmul(ps_ssr[:, :], sq0[:, :], ones_c_bf[:, :],
                                 start=True, stop=False)
                nc.tensor.matmul(ps_ssr[:, :], sq1[:, :],
                                 ones_c_bf[:QLP - 128, :],
                                 start=False, stop=True)

                # W2 row matmul -> partial q_abs rows [32, 9216 + 1]
                qrows = qwp.tile([B, NQX], BF16, tag="qrows")
                for nchi in range(NCH):
                    n0 = nchi * 512
                    ps_r = psqr.tile([B, 512], F32, tag="qr")
                    nc.tensor.matmul(ps_r[:, :], qaT0[:, :],
                                     w2a_sb[:, n0:n0 + 512],
                                     start=True, stop=False)
                    nc.tensor.matmul(ps_r[:, :], qaT1[:, :],
                                     w2b_sb[:, n0:n0 + 512],
                                     start=False, stop=True)
                    if nchi % 2 == 0:
                        nc.vector.tensor_copy(qrows[:, n0:n0 + 512],
                                              ps_r[:, :])
                    else:
                        nc.scalar.copy(qrows[:, n0:n0 + 512], ps_r[:, :])
                nc.vector.tensor_copy(qrows[:, NQ:NQX], ps_ssr[:, :])

                # ReduceScatter: sum partials, keep my 4 sequences
                rs_in = dramp.tile([B, NQX], BF16)
                rs_out = dramp.tile([BP, NQX], BF16)
                nc.sync.dma_start(rs_in[:, :], qrows[:, :])
                if fake_coll:
                    nc.sync.dma_start(rs_out[:, :], rs_in[0:BP, :])
                else:
                    nc.gpsimd.collective_compute(
                        "ReduceScatter", ADD, replica_groups=rg,
                        ins=[rs_in.opt()], outs=[rs_out.opt()])

            with tc.tile_pool(name="psqt", bufs=1, space="PSUM") as psqt:
                # reuses the qrows buffer (sequential lifetimes)
                qr4 = qwp.tile([B, NQX], BF16, tag="qrows", name="qr4")
                nc.sync.dma_start(qr4[:BP, :], rs_out[:, :])

                # rinv for my 4 seqs: exp(-0.5*ln(ms+eps)), broadcast to a
                # [128, 4] column tile (ln/exp/square/copy share one table)
                ps_sst = psqt.tile([1, BP], BF16, tag="sst")
                nc.tensor.transpose(ps_sst[:, :], qr4[:BP, NQ:NQX],
                                    ident[:BP, :BP])
                lnv = qwp.tile([1, BP], F32, tag="lnv")
                nc.scalar.activation(lnv[:, :], ps_sst[:, :], LN,
                                     bias=eps_t[:1, :1])
                rinv = qwp.tile([1, BP], F32, tag="rinv")
                nc.scalar.activation(rinv[:, :], lnv[:, :], EXP, scale=-0.5)
                ps_bc4 = psqt.tile([128, BP], F32, tag="bc4")
                nc.tensor.matmul(ps_bc4[:, :], ones_r_f[:1, :], rinv[:, :],
                                 start=True, stop=True)
                bc4 = qwp.tile([128, BP], BF16, tag="bc4s")
                nc.vector.tensor_copy(bc4[:, :], ps_bc4[:, :])

                # tiny transposes -> qabsT [c|128, (c h), b], qpeT [r, h, b]
                # (rinv applied during the PSUM->SBUF move)
                ps_qt = psqt.tile([128, H * 4 * BP], BF16, tag="qt")
                for g in range(H * 4):
                    nc.tensor.transpose(ps_qt[:, g * BP:(g + 1) * BP],
                                        qr4[:BP, g * 128:(g + 1) * 128],
                                        ident[:BP, :BP])
                nc.vector.tensor_tensor(
                    qabsT[:, :, :],
                    ps_qt[:, :].rearrange("p (g b) -> p g b", b=BP),
                    bc4[:, :].rearrange("p (o b) -> p o b", o=1)
                    .broadcast_to([128, H * 4, BP]), MULT)
                ps_qp = psqt.tile([DR, H * BP], BF16, tag="qp")
                for h in range(H):
                    nc.tensor.transpose(
                        ps_qp[:, h * BP:(h + 1) * BP],
                        qr4[:BP, NOPE + h * DR:NOPE + (h + 1) * DR],
                        ident[:BP, :BP])
                nc.vector.tensor_tensor(
                    qpeT[:, :, :],
                    ps_qp[:, :].rearrange("p (h b) -> p h b", b=BP),
                    bc4[:DR, :].rearrange("p (o b) -> p o b", o=1)
                    .broadcast_to([DR, H, BP]), MULT)
                if debug:
                    nc.sync.dma_start(dbg_qr4[:, :], qr4[:BP, 0:NQ])

            # ================= attention =================
            ctxT = atp.tile([128, 4, H, BP], BF16)
            wvc_sb = osb.tile([128, H * 4, DV], BF16)
            wo_sb = osb.tile([128, H, HO], BF16)

            with (
                tc.tile_pool(name="pssc", bufs=1, space="PSUM") as pssc,
                tc.tile_pool(name="pstr", bufs=2, space="PSUM") as pstr,
                tc.tile_pool(name="psmi", bufs=1, space="PSUM") as psmi,
            ):
                mct = 0  # natc copy rotation counter
                # one shared sums bank: per-seq chains in disjoint regions
                # open strictly one after another (complete before next opens)
                ps_sums = psmi.tile([1, BP * H], F32, tag="sums")

                for lb in range(BP):
                    if lb not in ctr_tiles:
                        issue_ctr(lb)
                    ps_sc = pssc.tile([128, 512], F32, tag="sc",
                                      name=f"sc{lb}")
                    # one accumulation chain per PSUM bank: interleaved
                    # starts within a bank abort each other's open group
                    ctx_c = [psmi.tile([128, H], F32, tag=f"ctx{c}",
                                       name=f"ctx{c}_{lb}")
                             for c in range(4)]
                    sums_ap = ps_sums[:1, lb * H:(lb + 1) * H]
                    eT = etp.tile([128, ST * 4, H], BF16, tag="eT")
                    ctr_sb = ctr_tiles[lb]
                    pending = []  # deferred (st, natc tiles) for ctx stage

                    def ctx_stage(stage, lb=lb, eT=eT, ctx_c=ctx_c,
                                  sums_ap=sums_ap):
                        st, nats = stage
                        for i in range(4):
                            g = st * 4 + i
                            for c in range(4):
                                nc.tensor.matmul(
                                    ctx_c[c][:, :],
                                    nats[i][:, c * 128:(c + 1) * 128],
                                    eT[:, g, :],
                                    start=(g == 0),
                                    stop=(st == ST - 1 and i == 3))
                            nc.tensor.matmul(
                                sums_ap[:, :], ones_c_bf[:, :1], eT[:, g, :],
                                start=(g == 0), stop=(g == ST * 4 - 1))

                    for st in range(ST):
                        if (lb, st) not in ctl_tiles:
                            issue_ctl(lb, st)
                        ctl = ctl_tiles.pop((lb, st))
                        # scores [s, 16h]: cache tile stationary, q moving
                        for i in range(4):
                            sc_ap = ps_sc[:, (st * 4 + i) * 16:
                                          (st * 4 + i + 1) * 16]
                            for c in range(4):
                                nc.tensor.matmul(
                                    sc_ap[:, :],
                                    ctl[:, c, i * 128:(i + 1) * 128],
                                    qabsT[:, c * 16:(c + 1) * 16, lb],
                                    start=(c == 0), stop=False)
                            nc.tensor.matmul(
                                sc_ap[:, :],
                                ctr_sb[:, st * 512 + i * 128:
                                       st * 512 + (i + 1) * 128],
                                qpeT[:, :, lb], start=False, stop=True)
                        # exp straight out of PSUM into bf16 e^T
                        nc.scalar.activation(
                            eT[:, st * 4:(st + 1) * 4, :],
                            ps_sc[:, st * 64:(st + 1) * 64]
                            .rearrange("p (i h) -> p i h", i=4),
                            EXP, scale=SCALE)
                        # natural tiles: 3 of 4 via PE transpose + spread
                        # copies; 1 of 4 streamed from the host natural
                        # layout (trades spare DMA for engine time)
                        nats = []
                        for i in range(4):
                            natc = natp.tile([128, 512], BF16, tag="nat")
                            if i == 3:
                                s0 = st * 512 + i * 128
                                nc.sync.dma_start(
                                    natc[:, :],
                                    cnat_d[lb, s0:s0 + 128, :])
                                nats.append(natc)
                                continue
                            ps_tr = pstr.tile([128, 512], BF16, tag="tr")
                            for c in range(4):
                                nc.tensor.transpose(
                                    ps_tr[:, c * 128:(c + 1) * 128],
                                    ctl[:, c, i * 128:(i + 1) * 128],
                                    ident[:, :])
                            r = mct % ncpy
                            mct += 1
                            if r < CPY[0]:
                                nc.vector.tensor_copy(natc[:, :], ps_tr[:, :])
                            else:
                                nc.scalar.copy(natc[:, :], ps_tr[:, :])
                            nats.append(natc)
                        # context deferred TWO stages so exp and the copies
                        # have slack before the PE consumes them
                        pending.append((st, nats))
                        if len(pending) > 2:
                            ctx_stage(pending.pop(0))
                    for stg in pending:
                        ctx_stage(stg)

                    # unnormalized ctx -> ctxT (softmax denom applied at the
                    # ovT stage, once for all 4 seqs)
                    for c in range(4):
                        if c % 2 == 0:
                            nc.vector.tensor_copy(ctxT[:, c, :, lb],
                                                  ctx_c[c][:, :])
                        else:
                            nc.scalar.copy(ctxT[:, c, :, lb], ctx_c[c][:, :])

                    if lb == 1:
                        nc.sync.dma_start(wvc_sb[:, :, :], wvc[:, :, :])
                    if lb == 2:
                        nc.sync.dma_start(wo_sb[:, :, :], wo[:, :, :])
                    if debug and lb == 0:
                        nc.sync.dma_start(dbg_et[:, :, :], eT[:, :, :])

                # reciprocal of all 32 softmax denominators, (b, h) order
                recip_sb = atp.tile([1, BP * H], F32)
                nc.vector.reciprocal(recip_sb[:, :], ps_sums[:1, :])

            # ================= output =================
            with (
                tc.tile_pool(name="psov", bufs=1, space="PSUM") as psov,
                tc.tile_pool(name="psoo", bufs=2, space="PSUM") as psoo,
            ):
                # softmax denominators broadcast down the partitions
                ps_rba = psov.tile([128, H, BP], F32, tag="rba")
                nc.tensor.matmul(
                    ps_rba[:, :, :], ones_r_f[:1, :],
                    recip_sb[:, :].rearrange("p (b h) -> p h b", h=H),
                    start=True, stop=True)
                rb_all = osb.tile([128, H, BP], BF16)
                nc.vector.tensor_copy(rb_all[:, :, :], ps_rba[:, :, :])

                # un-absorb values: ovT [v, h, b], normalized here
                ps_ov = psov.tile([128, H, BP], F32, tag="ov")
                for h in range(H):
                    for c in range(4):
                        nc.tensor.matmul(ps_ov[:, h, :],
                                         wvc_sb[:, h * 4 + c, :],
                                         ctxT[:, c, h, :],
                                         start=(c == 0), stop=(c == 3))
                ovT_sb = osb.tile([128, H, BP], BF16)
                nc.vector.tensor_tensor(ovT_sb[:, :, :], ps_ov[:, :, :],
                                        rb_all[:, :, :], MULT)

                # -> rows [4, 2048] -> AllGather -> [32, 2048]
                ps_or0 = psov.tile([BP, 8 * DV], BF16, tag="or0")
                ps_or1 = psov.tile([BP, 8 * DV], BF16, tag="or1")
                for h in range(H):
                    pst = ps_or0 if h < 8 else ps_or1
                    nc.tensor.transpose(
                        pst[:, (h % 8) * DV:(h % 8 + 1) * DV],
                        ovT_sb[:, h, :], ident[:, :])
                ovr = osb.tile([BP, H * DV], BF16)
                nc.vector.tensor_copy(ovr[:, 0:8 * DV], ps_or0[:, :])
                nc.vector.tensor_copy(ovr[:, 8 * DV:], ps_or1[:, :])
                if debug:
                    nc.sync.dma_start(dbg_ctxT[:, :, :, :], ctxT[:, :, :, :])
                    nc.sync.dma_start(dbg_ovr[:, :], ovr[:, :])
                ag_in = dramp.tile([BP, H * DV], BF16)
                ag_out = dramp.tile([B, H * DV], BF16)
                nc.sync.dma_start(ag_in[:, :], ovr[:, :])
                if fake_coll:
                    nc.sync.dma_start(ag_out[0:BP, :], ag_in[:, :])
                else:
                    nc.gpsimd.collective_compute(
                        "AllGather", BYPASS, replica_groups=rg,
                        ins=[ag_in.opt()], outs=[ag_out.opt()])
                ov32 = osb.tile([B, H * DV], BF16)
                nc.sync.dma_start(ov32[:, :], ag_out[:, :])
                ps_ot = psov.tile([128, H * B], BF16, tag="ot")
                for kt in range(H):
                    nc.tensor.transpose(ps_ot[:, kt * B:(kt + 1) * B],
                                        ov32[:B, kt * 128:(kt + 1) * 128],
                                        ident[:B, :B])
                ovT_all = osb.tile([128, H, B], BF16)
                nc.vector.tensor_copy(
                    ovT_all[:, :, :],
                    ps_ot[:, :].rearrange("p (k b) -> p k b", b=B))

                # output projection (columns n0..n0+640 of the full output)
                outT_sb = osb.tile([128, 5, B], F32)
                for n in range(5):
                    ps_o = psoo.tile([128, B], F32, tag="oo")
                    for kt in range(H):
                        nc.tensor.matmul(ps_o[:, :],
                                         wo_sb[:, kt, n * 128:(n + 1) * 128],
                                         ovT_all[:, kt, :],
                                         start=(kt == 0), stop=(kt == H - 1))
                    nc.vector.tensor_copy(outT_sb[:, n, :], ps_o[:, :])
                nc.sync.dma_start(
                    out[:, :].rearrange("(n p) b -> p n b", p=128),
                    outT_sb[:, :, :])

    nc.compile()
    return nc


# ----------------------------- host wrapper ------------------------------


def _prep_in_maps(inputs, S, n_cores):
    hidden = np.asarray(inputs["hidden_states"], np.float32)
    pos = np.asarray(inputs["positions"], np.int32)
    w_qkv_a = np.asarray(inputs["w_qkv_a"], np.float32)
    q_a_norm_w = np.asarray(inputs["q_a_norm_w"], np.float32)
    w_q_b = np.asarray(inputs["w_q_b"], np.float32)
    kv_a_norm_w = np.asarray(inputs["kv_a_norm_w"], np.float32)
    w_kc = np.asarray(inputs["w_kc"], np.float32)
    w_vc = np.asarray(inputs["w_vc"], np.float32)
    w_o = np.asarray(inputs["w_o"], np.float32)
    cache_l = np.asarray(inputs["kv_cache_latent"], np.float32)
    cache_r = np.asarray(inputs["kv_cache_rope"], np.float32)

    # current-token cache update (host)
    latent = hidden @ w_qkv_a[:, QL:QL + KL]
    k_pe = hidden @ w_qkv_a[:, QL + KL:]
    cache_l = cache_l.copy()
    cache_r = cache_r.copy()
    cache_l[:, -1, :] = _rmsnorm_np(latent, kv_a_norm_w)
    cache_r[:, -1, :] = _rope_np(k_pe.astype(np.float32), pos)

    # fold q-rope into the rope cache: k' = R(pos_b)^T k
    inv = (1.0 / (BASE ** (np.arange(0, DR, 2, dtype=np.float32) / DR)))
    fr = pos.astype(np.float32)[:, None] * inv.astype(np.float32)
    cos = np.cos(fr).astype(np.float32)[:, None, :]
    sin = np.sin(fr).astype(np.float32)[:, None, :]
    cr1, cr2 = cache_r[..., 0::2], cache_r[..., 1::2]
    cr_rot = np.empty_like(cache_r)
    cr_rot[..., 0::2] = cos * cr1 + sin * cr2
    cr_rot[..., 1::2] = -sin * cr1 + cos * cr2

    # packed transposed caches, bf16
    # ctl[b] = [p, t, s] with c = t*128+p ; ctr[b] = [r, s]
    ctl_all = np.ascontiguousarray(
        cache_l[:, :S, :].transpose(0, 2, 1).reshape(B, 4, 128, S)
        .transpose(0, 2, 1, 3)).astype(BF)
    ctr_all = np.ascontiguousarray(cr_rot[:, :S, :].transpose(0, 2, 1)
                                   ).astype(BF)
    cnat_all = cache_l[:, :S, :].astype(BF)

    # W2 = [absorbed nope (c-chunk, h, 128) | rope (h, r)], norm scale folded
    w_qb_eff = q_a_norm_w[:, None] * w_q_b
    wq = w_qb_eff.reshape(QL, H, DN + DR)
    w_nope, w_pe = wq[:, :, :DN], wq[:, :, DN:]
    Wabs = np.einsum("qhd,hdc->qhc", w_nope, w_kc, optimize=True)
    nope_cols = Wabs.reshape(QL, H, 4, 128).transpose(0, 2, 1, 3).reshape(
        QL, NOPE)
    W2 = np.concatenate([nope_cols, w_pe.reshape(QL, H * DR)], axis=1)

    hT_p = np.ascontiguousarray(
        hidden.T.reshape(KTH, 128, B).transpose(1, 0, 2)).astype(BF)
    wvc_p = np.ascontiguousarray(
        w_vc.reshape(H, 4, 128, DV).transpose(2, 0, 1, 3).reshape(
            128, H * 4, DV)).astype(BF)

    in_maps = []
    for k in range(n_cores):
        b0 = k * BP
        q0 = k * QLP
        wqa_p = np.ascontiguousarray(
            w_qkv_a[:, q0:q0 + QLP].reshape(KTH, 128, QLP)
            .transpose(1, 0, 2)).astype(BF)
        w2s = W2[q0:q0 + QLP, :]
        wo_p = np.ascontiguousarray(
            w_o[:, k * HO:(k + 1) * HO].reshape(H, 128, HO)
            .transpose(1, 0, 2)).astype(BF)
        m = {
            "hT": hT_p,
            "wqa": wqa_p,
            "w2a": np.ascontiguousarray(w2s[0:128, :]).astype(BF),
            "w2b": np.ascontiguousarray(w2s[128:QLP, :]).astype(BF),
            "ctl": np.ascontiguousarray(ctl_all[b0:b0 + BP]),
            "ctr": np.ascontiguousarray(ctr_all[b0:b0 + BP]),
            "cnat": np.ascontiguousarray(cnat_all[b0:b0 + BP]),
            "wvc": wvc_p,
            "wo": wo_p,
        }
        in_maps.append(m)
    return in_maps


def _unshard(results):
    return np.concatenate(
        [np.asarray(results[k]["out"], np.float32).T
         for k in range(N_CORES)], axis=1)


def run(inputs, S=4096, trace=False):
    key = (S, N_CORES)
    if key not in _CACHE:
        _CACHE[key] = _build(S, N_CORES)
    nc = _CACHE[key]
    in_maps = _prep_in_maps(inputs, S, N_CORES)
    res = bass_utils.run_bass_kernel_spmd(
        nc, in_maps, core_ids=list(range(N_CORES)), trace=trace)
    return _unshard(res.results), res


def kernel(**inputs) -> np.ndarray:
    out, _ = run(inputs)
    return out.astype(np.float32)
